# revision 1
# baseline (speedup 1.0000x reference)
"""Trainium2 Bass kernel for nn_DInPBlock (involution block, dense_cnn).

Sharding: pure data parallel - batch dim (8) across 8 NeuronCores, one
image per core. All weights/BN/PReLU params are host-folded and
replicated.

v2 pipeline (per core, image (32,256,256) -> (320,128,128)):
  - x loaded into row-parity banks, then column-parity split on GpSimd
    so every involution tap is a stride-1 slice (DVE 2x mode)
  - all layout transposes are single direct SBUF->SBUF DMAs (no DRAM
    bounce round-trips)
  - kernel-generation chains are PE matmuls (quadrant-packed) with
    BN+ReLU fused into Activation-engine PSUM->SBUF reads
  - 5 dilated branches software-pipelined (kgen of branch i+1 overlaps
    the DVE apply of branch i, chain of branch i-1 overlaps both)
  - BN+PReLU chains run on the Activation engine as two opposed ReLUs
    combined by one DVE scalar_tensor_tensor
  - output written bf16, host upcasts
"""

import numpy as np
import ml_dtypes
from contextlib import ExitStack

import concourse.bass as bass
import concourse.bacc as bacc
import concourse.tile as tile
import concourse.mybir as mybir
from concourse.bass_utils import run_bass_kernel_spmd

F32 = mybir.dt.float32
BF16 = mybir.dt.bfloat16
AF = mybir.ActivationFunctionType
OP = mybir.AluOpType

DILS = (1, 2, 4, 8, 16)
BRANCH_ORDER = (4, 3, 2, 1, 0)  # descending dilation (kih zero rows survive)
C1, C2, OH, OW = 32, 64, 128, 128
NPIX = OH * OW  # 16384
W2 = 160  # o1h padded width (16 + 128 + 16)
XW = 258  # x bank padded width (1 + 256 + 1)
PO = 129  # odd-column bank width


def _bn_fold(g, b, m, v, eps):
    sc = g / np.sqrt(v + eps)
    return sc.astype(np.float32), (b - m * sc).astype(np.float32)


def build():
    nc = bacc.Bacc("TRN2", target_bir_lowering=False, debug=False)

    x_d = nc.declare_dram_parameter("xin", [C1, 256, 256], BF16, isOutput=False).ap()
    w1red_d = nc.declare_dram_parameter("w1red_t", [128, C1], BF16, isOutput=False).ap()
    w1span_d = nc.declare_dram_parameter("w1span_t", [128, 32], BF16, isOutput=False).ap()
    w1init_d = nc.declare_dram_parameter("w1init_t", [128, C2], BF16, isOutput=False).ap()
    s1p_d = nc.declare_dram_parameter("s1p", [2 * C1, 1], F32, isOutput=False).ap()
    bn1p_d = nc.declare_dram_parameter("bn1p", [128, 4], F32, isOutput=False).ap()
    pr1n_d = nc.declare_dram_parameter("pr1n2", [128, 1], F32, isOutput=False).ap()
    wdred_d = nc.declare_dram_parameter("wdred_t", [128, 5 * C2], BF16, isOutput=False).ap()
    wdspan_d = nc.declare_dram_parameter("wdspan_t", [128, 5 * 32], BF16, isOutput=False).ap()
    sdp_d = nc.declare_dram_parameter("sdp", [2 * C2, 5], F32, isOutput=False).ap()
    chp_d = nc.declare_dram_parameter("chainp", [128, 5 * 6], F32, isOutput=False).ap()
    zbf_d = nc.declare_dram_parameter("zeros_bf", [16, C2 * W2], BF16, isOutput=False).ap()
    y_d = nc.declare_dram_parameter("yout", [5 * C2, NPIX], BF16, isOutput=True).ap()

    with tile.TileContext(nc) as tc, ExitStack() as top:
        pp = top.enter_context(tc.tile_pool(name="params", bufs=1))
        live = top.enter_context(tc.tile_pool(name="live", bufs=1))
        pbig = top.enter_context(tc.tile_pool(name="pbig", bufs=2, space="PSUM"))
        psmall = top.enter_context(tc.tile_pool(name="psmall", bufs=3, space="PSUM"))
        dp = top.enter_context(tc.tile_pool(name="dbounce", bufs=2, space="DRAM"))

        w1red_s = pp.tile([128, C1], BF16, tag="w1red")
        w1span_s = pp.tile([128, 32], BF16, tag="w1span")
        w1init_s = pp.tile([128, C2], BF16, tag="w1init")
        nc.sync.dma_start(w1red_s[:], w1red_d[:])
        nc.sync.dma_start(w1span_s[:], w1span_d[:])
        nc.sync.dma_start(w1init_s[:], w1init_d[:])
        s1p_s = pp.tile([2 * C1, 1], F32, tag="s1p")
        nc.sync.dma_start(s1p_s[:], s1p_d[:])
        s1sc_s, s1bi_s = s1p_s[0:C1], s1p_s[C1:2 * C1]
        bn1p_s = pp.tile([128, 4], F32, tag="bn1p")
        nc.sync.dma_start(bn1p_s[:], bn1p_d[:])
        bn1_s = [bn1p_s[:, j:j + 1] for j in range(4)]
        pr1n_s = pp.tile([128, 1], F32, tag="pr1n")
        nc.sync.dma_start(pr1n_s[:], pr1n_d[:])
        wdred_s = pp.tile([128, 5 * C2], BF16, tag="wdred")
        nc.sync.dma_start(wdred_s[:], wdred_d[:])
        wdspan_s = pp.tile([128, 5 * 32], BF16, tag="wdspan")
        nc.sync.dma_start(wdspan_s[:], wdspan_d[:])
        sdp_s = pp.tile([2 * C2, 5], F32, tag="sdp")
        nc.sync.dma_start(sdp_s[:], sdp_d[:])
        chp_s = pp.tile([128, 5 * 6], F32, tag="chp")
        nc.sync.dma_start(chp_s[:], chp_d[:])
        zt = pp.tile([16, 3 * OW], BF16, tag="zt")
        nc.gpsimd.memset(zt[:], 0.0)

        o1_c = live.tile([128, NPIX // 2], BF16, tag="o1c")
        o1h = live.tile([128, C2, W2], BF16, tag="o1h")

        # ================= invo1: pool, kgen1, apply, init conv =========
        with tc.tile_pool(name="parity", bufs=1) as parp:
            # quadrant-packed c-layout copy of x: partition 32q+c holds
            # x[c, 64q:64q+64, :] -- one DMA of 128 x 32KB packets. The 2x2
            # pool is computed directly from it, so kgen1 starts without
            # waiting for the h-layout parity banks or any bounce.
            xq = parp.tile([128, 64 * 256], BF16, tag="xq")
            xdv = x_d.rearrange("c (q rr) w -> q c (rr w)", q=4)
            for q in range(4):
                nc.sync.dma_start(xq[32 * q:32 * q + 32], xdv[q])
            xeE = parp.tile([128, C1, OW], BF16, tag="xeE")
            xeO = parp.tile([128, C1, PO], BF16, tag="xeO")
            xoE = parp.tile([128, C1, OW], BF16, tag="xoE")
            xoO = parp.tile([128, C1, PO], BF16, tag="xoO")
            xdE = parp.tile([128, C1, OW], BF16, tag="xdE")
            xdO = parp.tile([128, C1, PO], BF16, tag="xdO")

            with tc.tile_pool(name="xbank", bufs=1) as xp:
                xe = xp.tile([128, C1, XW], BF16, tag="xe")
                xo = xp.tile([128, C1, XW], BF16, tag="xo")
                xv = x_d.rearrange("c (h2 t) w -> t h2 c w", t=2)
                nc.sync.dma_start(xe[:, :, 1:257], xv[0])
                nc.sync.dma_start(xo[:, :, 1:257], xv[1])
                nc.vector.memset(xe[:, :, 0:1], 0.0)
                nc.vector.memset(xo[:, :, 0:1], 0.0)
                # parity split: pair p covers padded cols (2p, 2p+1)
                #   [..., 0] = x col 2p-1 (odd bank), [..., 1] = x col 2p (even)
                xev = xe.rearrange("p c (ow t) -> p c ow t", t=2)
                xov = xo.rearrange("p c (ow t) -> p c ow t", t=2)
                nc.gpsimd.tensor_copy(xeO[:], xev[:, :, 0:PO, 0])
                nc.gpsimd.tensor_copy(xeE[:], xev[:, :, 0:OW, 1])
                nc.vector.tensor_copy(xoO[:], xov[:, :, 0:PO, 0])
                nc.vector.tensor_copy(xoE[:], xov[:, :, 0:OW, 1])

            # xd = odd rows shifted down one output row (row -1 = zeros)
            nc.sync.dma_start(xdE[1:128], xoE[0:127])
            nc.sync.dma_start(xdO[1:128], xoO[0:127])
            nc.gpsimd.memset(xdE[0:1, :, :], 0.0)
            nc.gpsimd.memset(xdO[0:1, :, :], 0.0)

            with tc.tile_pool(name="sp1a", bufs=1) as sp1:
                # 2x2 pool sums in c-layout (scale 0.25 folded into w1red)
                o_c = sp1.tile([128, 32 * OW], BF16, tag="o_c")
                t1 = sp1.tile([128, C1, OW], BF16, tag="scrB")
                xqv = xq.rearrange("p (oh2 r ow u) -> p oh2 r ow u",
                                   oh2=32, r=2, ow=128, u=2)
                o_cv = o_c.rearrange("p (a b) -> p a b", a=32)
                nc.gpsimd.tensor_tensor(
                    t1[:], xqv[:, :, 0, :, 0], xqv[:, :, 0, :, 1], OP.add)
                nc.vector.tensor_tensor(
                    o_cv, xqv[:, :, 1, :, 0], xqv[:, :, 1, :, 1], OP.add)
                nc.gpsimd.tensor_tensor(o_cv, o_cv, t1[:], OP.add)

                # kgen1 reduce: r1 = relu(bn(w1red @ o))
                r1s = sp1.tile([128, 32 * OW], BF16, tag="r1s")
                for u in range(16):
                    q, uu = divmod(u, 4)
                    ps = pbig.tile([C2, 1024], F32, tag="mmbig")
                    for h in range(2):
                        nc.tensor.matmul(
                            ps[0:C1, h * 512:(h + 1) * 512],
                            w1red_s[32 * q:32 * q + 32],
                            o_c[32 * q:32 * q + 32,
                                uu * 1024 + h * 512: uu * 1024 + (h + 1) * 512],
                            tile_position=(32 * q, 0))
                    nc.scalar.activation(
                        r1s[32 * q:32 * q + 32, uu * 1024:(uu + 1) * 1024],
                        ps[0:C1], AF.Relu, bias=s1bi_s, scale=s1sc_s)

                # kgen1 span: k1 = w1span @ r1.  Partition block j holds
                # pixel rows [32j, 32j+32) so the h-layout transpose is a
                # 3-dim constant-stride AP per block.
                kstf1 = sp1.tile([128, 4096], BF16, tag="kstf1")
                for T in range(8):
                    ps2 = psmall.tile([128, 512], F32, tag="mmsmall")
                    for j in range(4):
                        nc.tensor.matmul(
                            ps2[32 * j:32 * j + 32],
                            w1span_s[32 * j:32 * j + 32],
                            r1s[32 * j:32 * j + 32, 512 * T:512 * (T + 1)],
                            tile_position=(32 * j, 32 * j))
                    nc.scalar.copy(kstf1[:, 512 * T:512 * (T + 1)], ps2[:])

                # k1 -> h-layout (via DRAM bounce, one read per row block)
                k1h = sp1.tile([128, 9, OW], BF16, tag="k1h")
                kb1 = dp.tile([128, 4096], BF16, tag="kb")
                nc.sync.dma_start(kb1[:], kstf1[:])
                k1v = kb1.rearrange("(j k) (r ow) -> j r k ow", j=4, k=32, r=32)
                for j in range(4):
                    nc.sync.dma_start(k1h[32 * j:32 * j + 32], k1v[j, :, 0:9])

                # invo1 apply (all taps stride-1 -> DVE 2x)
                acc1 = sp1.tile([128, C1, OW], BF16, tag="scrC")
                tmp1 = sp1.tile([128, C1, OW], BF16, tag="scrD")
                banks1 = ((xdO, xdE), (xeO, xeE), (xoO, xoE))
                for dy in range(3):
                    bO, bE = banks1[dy]
                    taps = (bO[:, :, 0:OW], bE[:, :, 0:OW], bO[:, :, 1:PO])
                    for dx in range(3):
                        k = 3 * dy + dx
                        in1 = k1h[:, k].unsqueeze(1).broadcast_to([128, C1, OW])
                        if k == 0:
                            nc.vector.tensor_tensor(acc1[:], taps[dx], in1, OP.mult)
                        else:
                            nc.vector.tensor_tensor(tmp1[:], taps[dx], in1, OP.mult)
                            nc.vector.tensor_tensor(acc1[:], acc1[:], tmp1[:], OP.add)

                appx_c = sp1.tile([128, 32 * OW], BF16, tag="appx")
                ab = dp.tile([C1, NPIX], BF16, tag="cb32")
                nc.sync.dma_start(
                    ab.rearrange("c (oh ow) -> oh c ow", ow=OW), acc1[:])
                for q in range(4):
                    nc.sync.dma_start(
                        appx_c[32 * q:32 * q + 32], ab[:, 4096 * q:4096 * (q + 1)])

                # init conv (32->64) + bn1 + prelu -> o1_c  (two col
                # halves, all on DVE so the Act queue stays clear for the
                # first branch's kgen)
                for h2 in range(2):
                    r1p = sp1.tile([128, 4096], BF16, tag="ract1")
                    r2p = sp1.tile([128, 4096], BF16, tag="ract2")
                    for u in (0, 1, 2, 3, 8, 9, 10, 11):
                        u += 4 * h2
                        q, uu = divmod(u, 4)
                        hh = u // 8
                        col0 = (u * 1024) % 4096
                        ps = pbig.tile([C2, 1024], F32, tag="mmbig")
                        for h in range(2):
                            nc.tensor.matmul(
                                ps[:, h * 512:(h + 1) * 512],
                                w1init_s[32 * q:32 * q + 32],
                                appx_c[32 * q:32 * q + 32,
                                       uu * 1024 + h * 512: uu * 1024 + (h + 1) * 512],
                                tile_position=(32 * q, 0))
                        nc.vector.tensor_scalar(
                            r1p[64 * hh:64 * hh + 64, col0:col0 + 1024],
                            ps[:], bn1p_s[64 * hh:64 * hh + 64, 0:1],
                            bn1p_s[64 * hh:64 * hh + 64, 1:2],
                            OP.mult, OP.add)
                    nc.vector.tensor_scalar_mul(r2p[:], r1p[:], pr1n_s[:])
                    nc.vector.tensor_tensor(
                        o1_c[:, 4096 * h2:4096 * (h2 + 1)],
                        r1p[:], r2p[:], OP.max)

        # o1 -> h-layout (padded), via DRAM bounce
        nc.vector.memset(o1h[:, :, 0:16], 0.0)
        nc.vector.memset(o1h[:, :, 144:160], 0.0)
        o1b = dp.tile([C2, NPIX], BF16, tag="cb64")
        for hh in range(2):
            nc.sync.dma_start(
                o1b[:, 8192 * hh:8192 * (hh + 1)], o1_c[64 * hh:64 * hh + 64])
        nc.sync.dma_start(
            o1h[:, :, 16:144], o1b.rearrange("c (oh ow) -> oh c ow", ow=OW))

        # ================= branches (software-pipelined) =================
        with tc.tile_pool(name="bsh", bufs=1) as bsh, \
             tc.tile_pool(name="bk", bufs=1) as bk, \
             tc.tile_pool(name="bkih", bufs=2) as bkih, \
             tc.tile_pool(name="bacc", bufs=1) as ba, \
             tc.tile_pool(name="bch", bufs=1) as bch:

            def stage_kgen(i):
                """kgen for branch i: ris, span, kih transpose."""
                ris = bk.tile([128, NPIX // 2], BF16, tag="ris")
                for u in range(16):
                    hh = u // 8
                    col0 = (u * 1024) % (NPIX // 2)
                    ps = pbig.tile([C2, 1024], F32, tag="mmbig")
                    for h in range(2):
                        nc.tensor.matmul(
                            ps[:, h * 512:(h + 1) * 512],
                            wdred_s[64 * hh:64 * hh + 64, i * C2:(i + 1) * C2],
                            o1_c[64 * hh:64 * hh + 64,
                                 col0 + h * 512:col0 + (h + 1) * 512],
                            tile_position=(64 * hh, 0))
                    nc.scalar.activation(
                        ris[64 * hh:64 * hh + 64, col0:col0 + 1024],
                        ps[:], AF.Relu,
                        bias=sdp_s[C2:2 * C2, i:i + 1], scale=sdp_s[0:C2, i:i + 1])

                # partition block j holds pixel rows [32j, 32j+32)
                kstf = bk.tile([128, 4096], BF16, tag="kstf")
                for T in range(8):
                    ps2 = psmall.tile([128, 512], F32, tag="mmsmall")
                    for j in range(4):
                        hh = j // 2
                        col0 = 4096 * (j % 2) + 512 * T
                        nc.tensor.matmul(
                            ps2[32 * j:32 * j + 32],
                            wdspan_s[64 * hh:64 * hh + 64, i * 32:(i + 1) * 32],
                            ris[64 * hh:64 * hh + 64, col0:col0 + 512],
                            tile_position=(64 * hh, 32 * j))
                    nc.scalar.copy(kstf[:, 512 * T:512 * (T + 1)], ps2[:])

                kih = bkih.tile([128, 9, OW], BF16, tag="kih")
                d = DILS[i]
                kb = dp.tile([128, 4096], BF16, tag="kb")
                nc.scalar.dma_start(kb[:], kstf[:])
                kv = kb.rearrange("(j k) (r ow) -> j r k ow", j=4, k=32, r=32)
                for j in range(4):
                    nc.scalar.dma_start(kih[32 * j:32 * j + 32], kv[j, :, 0:9])
                # zero the out-of-image rows of the edge-tap groups
                nc.scalar.dma_start(
                    kih[0:d, 0:3, :],
                    zt[0:d, 0:3 * OW].rearrange("p (k ow) -> p k ow", k=3))
                nc.scalar.dma_start(
                    kih[128 - d:128, 6:9, :],
                    zt[0:d, 0:3 * OW].rearrange("p (k ow) -> p k ow", k=3))
                return kih

            def stage_shifts(d):
                """row-shifted copies of o1h for the next apply."""
                o1u = bsh.tile([128, C2, W2], BF16, tag="o1u", bufs=2)
                nc.sync.dma_start(o1u[0:128 - d], o1h[d:128])
                nc.sync.dma_start(
                    o1u[128 - d:128],
                    zbf_d[0:d].rearrange("p (c w) -> p c w", c=C2))
                o1dn = bsh.tile([128, C2, W2], BF16, tag="o1dn", bufs=1)
                nc.sync.dma_start(o1dn[d:128], o1h[0:128 - d])
                nc.sync.dma_start(
                    o1dn[0:d], zbf_d[0:d].rearrange("p (c w) -> p c w", c=C2))
                return o1u, o1dn

            def stage_apply(d, kih, sh):
                """9-tap apply for branch with dilation d."""
                o1u, o1dn = sh
                acc = ba.tile([128, C2, OW], BF16, tag="acc", bufs=2)
                tmp = ba.tile([128, C2 // 2, OW], BF16, tag="tmp", bufs=1)
                first = True
                # o1dn group first so its WAR releases early (bufs=1)
                for bank, ks in ((o1dn, (0, 1, 2)), (o1u, (6, 7, 8)),
                                 (o1h, (3, 4, 5))):
                    for k in ks:
                        dx = k % 3
                        st = 16 + (dx - 1) * d
                        in0 = bank[:, :, st:st + OW]
                        in1 = kih[:, k].unsqueeze(1).broadcast_to([128, C2, OW])
                        if first:
                            nc.vector.tensor_tensor(acc[:], in0, in1, OP.mult)
                            first = False
                        else:
                            for ch in range(2):
                                cs = slice(32 * ch, 32 * ch + 32)
                                nc.vector.tensor_tensor(
                                    tmp[:], in0[:, cs], in1[:, cs], OP.mult)
                                nc.vector.tensor_tensor(
                                    acc[:, cs], acc[:, cs], tmp[:], OP.add)
                return acc

            def stage_yb(i, acc):
                yb = dp.tile([C2, NPIX], BF16, tag="yb")
                nc.sync.dma_start(
                    yb.rearrange("c (oh ow) -> oh c ow", ow=OW), acc[:])
                return yb

            def stage_chain(i, yb):
                """c-layout BN+PReLU x2 on DVE (TS at 4x / TT at 2x)."""
                cp = [chp_s[:, i * 6 + j:i * 6 + j + 1] for j in range(6)]
                ydv = y_d[i * C2:(i + 1) * C2, :].rearrange(
                    "c (hh g f) -> hh c g f", hh=2, g=2)
                for half in range(2):
                    ych = bch.tile([128, 4096], BF16, tag="ych", bufs=2)
                    for hh in range(2):
                        nc.sync.dma_start(
                            ych[64 * hh:64 * hh + 64],
                            yb[:, 8192 * hh + 4096 * half:
                               8192 * hh + 4096 * (half + 1)])
                    tA = bch.tile([128, 4096], BF16, tag="tA", bufs=1)
                    scr = bch.tile([128, 4096], BF16, tag="scr", bufs=1)
                    nc.vector.tensor_scalar(tA[:], ych[:], cp[0], cp[1],
                                            OP.mult, OP.add)
                    nc.vector.tensor_scalar_mul(scr[:], tA[:], cp[2])
                    nc.vector.tensor_tensor(ych[:], tA[:], scr[:], OP.max)
                    nc.vector.tensor_scalar(tA[:], ych[:], cp[3], cp[4],
                                            OP.mult, OP.add)
                    nc.vector.tensor_scalar_mul(scr[:], tA[:], cp[5])
                    nc.vector.tensor_tensor(ych[:], tA[:], scr[:], OP.max)
                    yov = ych.rearrange("(hh c) f -> hh c f", hh=2)
                    for hh in range(2):
                        nc.gpsimd.dma_start(ydv[hh, :, half], yov[hh])

            kihs = {}
            ybs = {}
            kihs[0] = stage_kgen(BRANCH_ORDER[0])
            sh = stage_shifts(DILS[BRANCH_ORDER[0]])
            for idx in range(5):
                if idx + 1 < 5:
                    kihs[idx + 1] = stage_kgen(BRANCH_ORDER[idx + 1])
                acc = stage_apply(DILS[BRANCH_ORDER[idx]], kihs.pop(idx), sh)
                if idx + 1 < 5:
                    sh = stage_shifts(DILS[BRANCH_ORDER[idx + 1]])
                ybs[idx] = stage_yb(BRANCH_ORDER[idx], acc)
                if idx > 0:
                    stage_chain(BRANCH_ORDER[idx - 1], ybs.pop(idx - 1))
            stage_chain(BRANCH_ORDER[4], ybs.pop(4))
    return nc


def prepare_inputs(inputs):
    """Host-side folding of all the small parameters; returns the in_map
    shared structure (everything except per-core x)."""
    f = lambda a: np.asarray(a, dtype=np.float32)
    m = {}
    m["w1red_t"] = np.ascontiguousarray(
        np.tile(f(inputs["w1_red"]).T * 0.25, (4, 1))).astype(ml_dtypes.bfloat16)
    w1s = np.zeros((C1, 32), np.float32)
    w1s[:, 0:9] = f(inputs["w1_span"]).T
    m["w1span_t"] = np.ascontiguousarray(np.tile(w1s, (4, 1))).astype(ml_dtypes.bfloat16)
    m["w1init_t"] = np.ascontiguousarray(
        np.tile(f(inputs["w1_init"]).T, (4, 1))).astype(ml_dtypes.bfloat16)
    s1sc, s1bi = _bn_fold(f(inputs["s1_g"]), f(inputs["s1_b"]),
                          f(inputs["s1_m"]), f(inputs["s1_v"]), 1e-5)
    m["s1p"] = np.concatenate([s1sc, s1bi]).reshape(2 * C1, 1)
    sc1, bi1 = _bn_fold(f(inputs["bn1_g"]), f(inputs["bn1_b"]),
                        f(inputs["bn1_m"]), f(inputs["bn1_v"]), 1e-3)
    m["bn1p"] = np.ascontiguousarray(
        np.tile(np.stack([sc1, bi1, -sc1, -bi1], axis=1), (2, 1)))
    m["pr1n2"] = np.tile(f(inputs["pr1"]), 2).reshape(128, 1)
    m["wdred_t"] = np.ascontiguousarray(np.tile(np.concatenate(
        [f(inputs["wd_red"])[i].T for i in range(5)], axis=1), (2, 1))
    ).astype(ml_dtypes.bfloat16)
    wds = np.zeros((C2, 5 * 32), np.float32)
    for i in range(5):
        wds[:, i * 32:i * 32 + 9] = f(inputs["wd_span"])[i].T
    m["wdspan_t"] = np.ascontiguousarray(np.tile(wds, (2, 1))).astype(ml_dtypes.bfloat16)
    sdsc, sdbi = _bn_fold(f(inputs["sd_g"]), f(inputs["sd_b"]),
                          f(inputs["sd_m"]), f(inputs["sd_v"]), 1e-5)
    m["sdp"] = np.ascontiguousarray(
        np.concatenate([sdsc, sdbi], axis=1).T)  # (2*C2, 5)
    bdsc, bdbi = _bn_fold(f(inputs["bnd_g"]), f(inputs["bnd_b"]),
                          f(inputs["bnd_m"]), f(inputs["bnd_v"]), 1e-3)
    bfsc_all, bfbi_all = _bn_fold(f(inputs["bnf_g"]), f(inputs["bnf_b"]),
                                  f(inputs["bnf_m"]), f(inputs["bnf_v"]), 1e-3)
    ch = np.zeros((128, 5 * 6), np.float32)
    t2 = lambda a: np.tile(a, 2)
    for i in range(5):
        scd, bid = bdsc[i], bdbi[i]
        ad = f(inputs["prd"])[i]
        scf = bfsc_all[i * C2:(i + 1) * C2]
        bif = bfbi_all[i * C2:(i + 1) * C2]
        af = f(inputs["prf"])[i * C2:(i + 1) * C2]
        cols = [scd, bid, ad, scf, bif, af]
        for j, v in enumerate(cols):
            ch[:, i * 6 + j] = t2(v)
    m["chainp"] = ch
    m["zeros_bf"] = np.zeros((16, C2 * W2), ml_dtypes.bfloat16)
    return m


_NC_CACHE = {}


def get_nc():
    if "nc" not in _NC_CACHE:
        nc = build()
        nc.compile()
        _NC_CACHE["nc"] = nc
    return _NC_CACHE["nc"]


def kernel(**inputs):
    nc = get_nc()
    shared = prepare_inputs(inputs)
    x = np.asarray(inputs["x"], dtype=np.float32)
    B = x.shape[0]
    in_maps = []
    for b in range(B):
        im = dict(shared)
        im["xin"] = np.ascontiguousarray(x[b]).astype(ml_dtypes.bfloat16)
        in_maps.append(im)
    res = run_bass_kernel_spmd(nc, in_maps, list(range(B)))
    out = np.stack([np.asarray(res.results[b]["yout"], dtype=np.float32)
                    .reshape(5 * C2, OH, OW) for b in range(B)], axis=0)
    return out


if __name__ == "__main__":
    # quick CoreSim check of core-0 program against numpy reference
    import reference as ref
    from concourse.bass_interp import CoreSim

    inputs = {k: np.asarray(v) for k, v in ref.setup_inputs().items()}
    expected = np.asarray(ref.reference(**inputs))
    nc = build()
    nc.compile()
    shared = prepare_inputs(inputs)
    sim = CoreSim(nc)
    for k, v in shared.items():
        sim.tensor(k)[:] = v
    sim.tensor("xin")[:] = np.asarray(inputs["x"][0]).astype(ml_dtypes.bfloat16)
    sim.simulate()
    got = np.array(sim.tensor("yout")).astype(np.float32).reshape(320, 128, 128)
    e = expected[0]
    err = np.linalg.norm(got - e) / np.linalg.norm(e)
    print("CoreSim core-0 relative error:", err)



# revision 5
# speedup vs baseline: 1.1151x; 1.1151x over previous
"""Trainium2 Bass kernel for nn_DInPBlock (involution block, dense_cnn).

Sharding: pure data parallel - batch dim (8) across 8 NeuronCores, one
image per core. All weights/BN/PReLU params are host-folded and
replicated.

v3 pipeline (per core, image (32,256,256) -> (320,128,128)):
  - BN+PReLU chains run as single Activation-engine Prelu ops with
    per-partition scale/bias/alpha (DVE does only the 9-tap applies)
  - kernel-generation matmuls are block-diagonal full-128-contraction
    PE ops (4x fewer instructions than quadrant-serial)
  - row-shifted apply banks (o1u/o1dn) are persistent pre-zeroed
    buffers; branches processed in descending dilation so the zero
    tails survive; no per-branch zero-fill DMAs
  - apply reads the unshifted o1h tap group first so the single-buffer
    shift DMAs for the next branch overlap the current apply
  - DMA traffic spread across the sync/scalar/gpsimd rings
"""

import numpy as np
import ml_dtypes
from contextlib import ExitStack

import concourse.bass as bass
import concourse.bacc as bacc
import concourse.tile as tile
import concourse.mybir as mybir
from concourse.bass_utils import run_bass_kernel_spmd

F32 = mybir.dt.float32
BF16 = mybir.dt.bfloat16
AF = mybir.ActivationFunctionType
OP = mybir.AluOpType

DILS = (1, 2, 4, 8, 16)
BRANCH_ORDER = (4, 3, 2, 1, 0)  # descending dilation (zero tails survive)
C1, C2, OH, OW = 32, 64, 128, 128
NPIX = OH * OW  # 16384
W2 = 160  # o1h padded width (16 + 128 + 16)
XW = 258  # x bank padded width (1 + 256 + 1)
PO = 129  # odd-column bank width


def _bn_fold(g, b, m, v, eps):
    sc = g / np.sqrt(v + eps)
    return sc.astype(np.float32), (b - m * sc).astype(np.float32)


def build():
    nc = bacc.Bacc("TRN2", target_bir_lowering=False, debug=False)

    x_d = nc.declare_dram_parameter("xin", [C1, 256, 256], BF16, isOutput=False).ap()
    w1red_d = nc.declare_dram_parameter("w1red_bd", [128, 128], BF16, isOutput=False).ap()
    w1span_d = nc.declare_dram_parameter("w1span_bd", [128, 128], BF16, isOutput=False).ap()
    w1init_d = nc.declare_dram_parameter("w1init_bd", [128, 128], BF16, isOutput=False).ap()
    s1p_d = nc.declare_dram_parameter("s1p4", [128, 2], F32, isOutput=False).ap()
    bn1p_d = nc.declare_dram_parameter("bn1p2", [128, 3], F32, isOutput=False).ap()
    wdred_d = nc.declare_dram_parameter("wdred_bd", [128, 5 * 128], BF16, isOutput=False).ap()
    wdspan_d = nc.declare_dram_parameter("wdspan_bd", [128, 5 * 64], BF16, isOutput=False).ap()
    sdp_d = nc.declare_dram_parameter("sdp4", [128, 10], F32, isOutput=False).ap()
    chp_d = nc.declare_dram_parameter("chainp", [128, 5 * 6], F32, isOutput=False).ap()
    y_d = nc.declare_dram_parameter("yout", [5 * C2, NPIX], BF16, isOutput=True).ap()

    with tile.TileContext(nc) as tc, ExitStack() as top:
        pp = top.enter_context(tc.tile_pool(name="params", bufs=1))
        lcx = top.enter_context(tc.tile_pool(name="lcx", bufs=1))
        pmm = top.enter_context(tc.tile_pool(name="pmm", bufs=2, space="PSUM"))
        pms = top.enter_context(tc.tile_pool(name="pms", bufs=2, space="PSUM"))
        dp = top.enter_context(tc.tile_pool(name="dbounce", bufs=2, space="DRAM"))

        w1red_s = pp.tile([128, 128], BF16, tag="w1red")
        w1span_s = pp.tile([128, 128], BF16, tag="w1span")
        w1init_s = pp.tile([128, 128], BF16, tag="w1init")
        nc.sync.dma_start(w1red_s[:], w1red_d[:])
        nc.sync.dma_start(w1span_s[:], w1span_d[:])
        nc.sync.dma_start(w1init_s[:], w1init_d[:])
        s1p_s = pp.tile([128, 2], F32, tag="s1p")
        nc.sync.dma_start(s1p_s[:], s1p_d[:])
        bn1p_s = pp.tile([128, 3], F32, tag="bn1p")
        nc.sync.dma_start(bn1p_s[:], bn1p_d[:])
        wdred_s = pp.tile([128, 5 * 128], BF16, tag="wdred")
        nc.sync.dma_start(wdred_s[:], wdred_d[:])
        wdspan_s = pp.tile([128, 5 * 64], BF16, tag="wdspan")
        nc.sync.dma_start(wdspan_s[:], wdspan_d[:])
        sdp_s = pp.tile([128, 10], F32, tag="sdp")
        nc.sync.dma_start(sdp_s[:], sdp_d[:])
        chp_s = pp.tile([128, 5 * 6], F32, tag="chp")
        nc.sync.dma_start(chp_s[:], chp_d[:])

        # o1 in c-layout: partition (64a + c), free (4096e + 512m + v);
        # pixel quadrant Q = h//32 maps to (a, e) = (Q%2, Q//2).
        o1cx = lcx.tile([128, 8192], BF16, tag="o1cx")

        # ================= invo1 ====================================
        with tc.tile_pool(name="sp1", bufs=1) as sp1:
            o_c = sp1.tile([128, 32 * OW], BF16, tag="o_c")
            r1s = sp1.tile([128, 4096], BF16, tag="r1s")
            kstf1 = sp1.tile([128, 4096], BF16, tag="kstf1")
            k1h = sp1.tile([128, 9, OW], BF16, tag="k1h")
            acc1 = sp1.tile([128, C1, OW], BF16, tag="acc1")
            tmp1 = sp1.tile([128, C1, OW], BF16, tag="tmp1")
            appx_c = sp1.tile([128, 4096], BF16, tag="appx")

            with tc.tile_pool(name="parity", bufs=1) as parp:
                # quadrant-packed c-layout copy of x: partition 32q+c
                # holds x[c, 64q:64q+64, :]
                xq = parp.tile([128, 64 * 256], BF16, tag="xq")
                xdv = x_d.rearrange("c (q rr) w -> q c (rr w)", q=4)
                for q in range(4):
                    nc.sync.dma_start(xq[32 * q:32 * q + 32], xdv[q])
                # row-parity banks (h-layout, padded cols)
                xe = parp.tile([128, C1, XW], BF16, tag="xe")
                xo = parp.tile([128, C1, XW], BF16, tag="xo")
                xv = x_d.rearrange("c (h2 t) w -> t h2 c w", t=2)
                nc.scalar.dma_start(xe[:, :, 1:257], xv[0])
                nc.scalar.dma_start(xo[:, :, 1:257], xv[1])
                nc.gpsimd.memset(xe[:, :, 0:1], 0.0)
                nc.gpsimd.memset(xo[:, :, 0:1], 0.0)
                xeE = parp.tile([128, C1, OW], BF16, tag="xeE")
                xeO = parp.tile([128, C1, PO], BF16, tag="xeO")
                xoE = parp.tile([128, C1, OW], BF16, tag="xoE")
                xoO = parp.tile([128, C1, PO], BF16, tag="xoO")
                xdE = parp.tile([128, C1, OW], BF16, tag="xdE")
                xdO = parp.tile([128, C1, PO], BF16, tag="xdO")

                # 2x2 pool sums from xq (scale folded into w1red)
                t1 = parp.tile([128, 32 * OW], BF16, tag="pt1")
                xqv = xq.rearrange("p (oh2 r ow u) -> p oh2 r ow u",
                                   oh2=32, r=2, ow=128, u=2)
                o_cv = o_c.rearrange("p (a b) -> p a b", a=32)
                t1v = t1.rearrange("p (a b) -> p a b", a=32)
                nc.gpsimd.tensor_tensor(
                    t1v, xqv[:, :, 0, :, 0], xqv[:, :, 0, :, 1], OP.add)
                nc.vector.tensor_tensor(
                    o_cv, xqv[:, :, 1, :, 0], xqv[:, :, 1, :, 1], OP.add)
                nc.vector.tensor_tensor(o_c[:], o_c[:], t1[:], OP.add)

                # column-parity split (strided copies on Act + Pool)
                xev = xe.rearrange("p c (ow t) -> p c ow t", t=2)
                xov = xo.rearrange("p c (ow t) -> p c ow t", t=2)
                nc.scalar.copy(xeO[:], xev[:, :, 0:PO, 0])
                nc.scalar.copy(xeE[:], xev[:, :, 0:OW, 1])
                nc.gpsimd.tensor_copy(xoO[:], xov[:, :, 0:PO, 0])
                nc.gpsimd.tensor_copy(xoE[:], xov[:, :, 0:OW, 1])

                # xd = odd rows shifted down one output row (row -1 = 0)
                nc.vector.memset(xdE[0:1], 0.0)
                nc.vector.memset(xdO[0:1], 0.0)
                nc.sync.dma_start(xdE[1:128], xoE[0:127])
                nc.sync.dma_start(xdO[1:128], xoO[0:127])

                # kgen1 reduce: r1 = relu(bn(w1red_bd @ o_c)) block-diag
                for t in range(4):
                    ps = pmm.tile([128, 1024], F32, tag="mm1")
                    for h in range(2):
                        nc.tensor.matmul(
                            ps[:, 512 * h:512 * (h + 1)],
                            w1red_s[:],
                            o_c[:, 1024 * t + 512 * h:1024 * t + 512 * (h + 1)])
                    nc.scalar.activation(
                        r1s[:, 1024 * t:1024 * (t + 1)], ps[:], AF.Relu,
                        bias=s1p_s[:, 1:2], scale=s1p_s[:, 0:1])

                # kgen1 span: k1 = w1span_bd @ r1 (block-diag); partition
                # block J holds pixel rows [32J, 32J+32)
                for t in range(8):
                    ps2 = pms.tile([128, 512], F32, tag="mm2")
                    nc.tensor.matmul(
                        ps2[:], w1span_s[:], r1s[:, 512 * t:512 * (t + 1)])
                    nc.scalar.copy(kstf1[:, 512 * t:512 * (t + 1)], ps2[:])

                # k1 -> h-layout via DRAM bounce
                kb1 = dp.tile([4, 9, 4096], BF16, tag="kb1", bufs=1)
                for j in range(4):
                    nc.scalar.dma_start(kb1[j], kstf1[32 * j:32 * j + 9])
                for j in range(4):
                    nc.scalar.dma_start(
                        k1h[32 * j:32 * j + 32],
                        kb1[j].rearrange("k (r ow) -> r k ow", r=32))

                # invo1 apply (all taps stride-1)
                banks1 = ((xdO, xdE), (xeO, xeE), (xoO, xoE))
                first = True
                for dy in range(3):
                    bO, bE = banks1[dy]
                    taps = (bO[:, :, 0:OW], bE[:, :, 0:OW], bO[:, :, 1:PO])
                    for dx in range(3):
                        k = 3 * dy + dx
                        in1 = k1h[:, k].unsqueeze(1).broadcast_to([128, C1, OW])
                        if first:
                            nc.vector.tensor_tensor(acc1[:], taps[dx], in1, OP.mult)
                            first = False
                        else:
                            nc.vector.tensor_tensor(tmp1[:], taps[dx], in1, OP.mult)
                            nc.vector.tensor_tensor(acc1[:], acc1[:], tmp1[:], OP.add)

            # apply1 output -> c-layout (quadrant packed) via DRAM bounce
            ab = dp.tile([C1, NPIX], BF16, tag="ab", bufs=1)
            nc.gpsimd.dma_start(
                ab.rearrange("c (oh ow) -> oh c ow", ow=OW), acc1[:])
            for q in range(4):
                nc.gpsimd.dma_start(
                    appx_c[32 * q:32 * q + 32], ab[:, 4096 * q:4096 * (q + 1)])

            # init conv (32->64) + bn1 + prelu -> o1cx, all fused on Act
            for e in range(2):
                for m in range(8):
                    ps = pms.tile([128, 512], F32, tag="mm2")
                    nc.tensor.matmul(
                        ps[:], w1init_s[64 * e:64 * e + 64],
                        appx_c[64 * e:64 * e + 64, 512 * m:512 * (m + 1)])
                    nc.scalar.activation(
                        o1cx[:, 4096 * e + 512 * m:4096 * e + 512 * (m + 1)],
                        ps[:], AF.Prelu,
                        bias=bn1p_s[:, 1:2], scale=bn1p_s[:, 0:1],
                        alpha=bn1p_s[:, 2:3])

        # ================= o1h + shifted banks =======================
        lv2 = top.enter_context(tc.tile_pool(name="lv2", bufs=1))
        o1h = lv2.tile([128, C2, W2], BF16, tag="o1h")
        o1u = lv2.tile([128, C2, W2], BF16, tag="o1u")
        o1dn = lv2.tile([128, C2, W2], BF16, tag="o1dn")

        # o1cx -> h-layout via DRAM bounce
        o1b = dp.tile([C2, NPIX], BF16, tag="o1b", bufs=1)
        for a in range(2):
            for e in range(2):
                nc.sync.dma_start(
                    o1b[:, 4096 * (2 * e + a):4096 * (2 * e + a + 1)],
                    o1cx[64 * a:64 * a + 64, 4096 * e:4096 * (e + 1)])
        nc.sync.dma_start(
            o1h[:, :, 16:144], o1b.rearrange("c (oh ow) -> oh c ow", ow=OW))
        # one-time zero pads / tails (Pool engine; branches descend in d)
        nc.gpsimd.memset(o1h[:, :, 0:16], 0.0)
        nc.gpsimd.memset(o1h[:, :, 144:160], 0.0)
        nc.gpsimd.memset(o1u[96:128], 0.0)
        nc.gpsimd.memset(o1dn[0:32], 0.0)

        # ================= branches (software-pipelined) =============
        with tc.tile_pool(name="bk", bufs=1) as bk, \
             tc.tile_pool(name="bkih", bufs=2) as bkih, \
             tc.tile_pool(name="bacc", bufs=1) as ba, \
             tc.tile_pool(name="bch", bufs=1) as bch:

            def stage_kgen(i):
                """kgen for branch i: ris, span, kih transpose."""
                ris = bk.tile([128, 8192], BF16, tag="ris")
                for t in range(8):
                    ps = pmm.tile([128, 1024], F32, tag="mm1")
                    for h in range(2):
                        nc.tensor.matmul(
                            ps[:, 512 * h:512 * (h + 1)],
                            wdred_s[:, 128 * i:128 * (i + 1)],
                            o1cx[:, 1024 * t + 512 * h:1024 * t + 512 * (h + 1)])
                    nc.scalar.activation(
                        ris[:, 1024 * t:1024 * (t + 1)], ps[:], AF.Relu,
                        bias=sdp_s[:, 2 * i + 1:2 * i + 2],
                        scale=sdp_s[:, 2 * i:2 * i + 1])

                kstf = bk.tile([128, 4096], BF16, tag="kstf")
                for m in range(8):
                    ps2 = pms.tile([128, 512], F32, tag="mm2")
                    for e in range(2):
                        nc.tensor.matmul(
                            ps2[64 * e:64 * e + 64],
                            wdspan_s[:, 64 * i:64 * (i + 1)],
                            ris[:, 4096 * e + 512 * m:4096 * e + 512 * (m + 1)],
                            tile_position=(0, 64 * e))
                    nc.scalar.copy(kstf[:, 512 * m:512 * (m + 1)], ps2[:])

                kih = bkih.tile([128, 9, OW], BF16, tag="kih")
                kb = dp.tile([4, 9, 4096], BF16, tag="kb")
                for j in range(4):
                    nc.scalar.dma_start(kb[j], kstf[32 * j:32 * j + 9])
                for j in range(4):
                    nc.scalar.dma_start(
                        kih[32 * j:32 * j + 32],
                        kb[j].rearrange("k (r ow) -> r k ow", r=32))
                return kih

            def stage_shifts(d):
                """refresh the persistent shifted banks for dilation d
                (descending d keeps the zero tails valid)."""
                nc.sync.dma_start(o1dn[d:128], o1h[0:128 - d])
                nc.sync.dma_start(o1u[0:128 - d], o1h[d:128])

            def stage_apply(d, kih):
                """9-tap apply; o1h group first so the single-buffered
                shift DMAs for the next branch overlap this apply."""
                acc = ba.tile([128, C2, OW], BF16, tag="acc", bufs=2)
                tmp = ba.tile([128, C2, OW], BF16, tag="tmp", bufs=1)
                first = True
                for bank, ks in ((o1h, (3, 4, 5)), (o1dn, (0, 1, 2)),
                                 (o1u, (6, 7, 8))):
                    for k in ks:
                        dx = k % 3
                        st = 16 + (dx - 1) * d
                        in0 = bank[:, :, st:st + OW]
                        in1 = kih[:, k].unsqueeze(1).broadcast_to([128, C2, OW])
                        if first:
                            nc.vector.tensor_tensor(acc[:], in0, in1, OP.mult)
                            first = False
                        else:
                            nc.vector.tensor_tensor(tmp[:], in0, in1, OP.mult)
                            nc.vector.tensor_tensor(acc[:], acc[:], tmp[:], OP.add)
                return acc

            def stage_yb(i, acc):
                yb = dp.tile([C2, NPIX], BF16, tag="yb")
                ybv = yb.rearrange("c (hh r ow) -> hh r c ow", hh=2, r=64)
                accv = acc.rearrange("(hh r) c ow -> hh r c ow", hh=2)
                for hh in range(2):
                    nc.gpsimd.dma_start(ybv[hh], accv[hh])
                return yb

            def stage_chain(i, yb):
                """chain = prelu(bnf(prelu(bnd(y)))) as two Act Prelus."""
                cp = [chp_s[:, i * 6 + j:i * 6 + j + 1] for j in range(6)]
                ydv = y_d[i * C2:(i + 1) * C2, :].rearrange(
                    "c (hh g f) -> hh c g f", hh=2, g=2)
                for half in range(2):
                    ya = bch.tile([128, 4096], BF16, tag="ya", bufs=2)
                    tb = bch.tile([128, 4096], BF16, tag="tb", bufs=1)
                    for hh in range(2):
                        nc.sync.dma_start(
                            ya[64 * hh:64 * hh + 64],
                            yb[:, 8192 * hh + 4096 * half:
                               8192 * hh + 4096 * (half + 1)])
                    nc.scalar.activation(tb[:], ya[:], AF.Prelu,
                                         bias=cp[1], scale=cp[0], alpha=cp[2])
                    nc.scalar.activation(ya[:], tb[:], AF.Prelu,
                                         bias=cp[4], scale=cp[3], alpha=cp[5])
                    yav = ya.rearrange("(hh c) f -> hh c f", hh=2)
                    for hh in range(2):
                        nc.gpsimd.dma_start(ydv[hh, :, half], yav[hh])

            kihs = {}
            ybs = {}
            kihs[0] = stage_kgen(BRANCH_ORDER[0])
            stage_shifts(DILS[BRANCH_ORDER[0]])
            kihs[1] = stage_kgen(BRANCH_ORDER[1])
            for idx in range(5):
                acc = stage_apply(DILS[BRANCH_ORDER[idx]], kihs.pop(idx))
                ybs[idx] = stage_yb(BRANCH_ORDER[idx], acc)
                if idx + 1 < 5:
                    stage_shifts(DILS[BRANCH_ORDER[idx + 1]])
                if idx + 2 < 5:
                    kihs[idx + 2] = stage_kgen(BRANCH_ORDER[idx + 2])
                stage_chain(BRANCH_ORDER[idx], ybs.pop(idx))
    return nc


def prepare_inputs(inputs):
    """Host-side folding of all the small parameters; returns the in_map
    shared structure (everything except per-core x)."""
    f = lambda a: np.asarray(a, dtype=np.float32)
    m = {}
    # block-diagonal weights
    w1red = np.zeros((128, 128), np.float32)
    w1span = np.zeros((128, 128), np.float32)
    wr = f(inputs["w1_red"]).T * 0.25  # [ci, co]
    ws = np.zeros((C1, 32), np.float32)
    ws[:, 0:9] = f(inputs["w1_span"]).T  # [j, k]
    for q in range(4):
        w1red[32 * q:32 * q + 32, 32 * q:32 * q + 32] = wr
        w1span[32 * q:32 * q + 32, 32 * q:32 * q + 32] = ws
    m["w1red_bd"] = w1red.astype(ml_dtypes.bfloat16)
    m["w1span_bd"] = w1span.astype(ml_dtypes.bfloat16)
    w1init = np.zeros((128, 128), np.float32)
    wi = f(inputs["w1_init"]).T  # [ci, co]
    for e in range(2):
        for a in range(2):
            w1init[64 * e + 32 * a:64 * e + 32 * a + 32,
                   64 * a:64 * a + 64] = wi
    m["w1init_bd"] = w1init.astype(ml_dtypes.bfloat16)

    s1sc, s1bi = _bn_fold(f(inputs["s1_g"]), f(inputs["s1_b"]),
                          f(inputs["s1_m"]), f(inputs["s1_v"]), 1e-5)
    m["s1p4"] = np.stack([np.tile(s1sc, 4), np.tile(s1bi, 4)], axis=1)
    sc1, bi1 = _bn_fold(f(inputs["bn1_g"]), f(inputs["bn1_b"]),
                        f(inputs["bn1_m"]), f(inputs["bn1_v"]), 1e-3)
    m["bn1p2"] = np.stack([np.tile(sc1, 2), np.tile(bi1, 2),
                           np.tile(f(inputs["pr1"]), 2)], axis=1)

    wdred = np.zeros((128, 5 * 128), np.float32)
    wdspan = np.zeros((128, 5 * 64), np.float32)
    for i in range(5):
        wrd = f(inputs["wd_red"])[i].T  # [ci, co]
        wsd = np.zeros((C2, 32), np.float32)
        wsd[:, 0:9] = f(inputs["wd_span"])[i].T  # [j, k]
        for a in range(2):
            wdred[64 * a:64 * a + 64, 128 * i + 64 * a:128 * i + 64 * a + 64] = wrd
            wdspan[64 * a:64 * a + 64, 64 * i + 32 * a:64 * i + 32 * a + 32] = wsd
    m["wdred_bd"] = wdred.astype(ml_dtypes.bfloat16)
    m["wdspan_bd"] = wdspan.astype(ml_dtypes.bfloat16)

    sdsc, sdbi = _bn_fold(f(inputs["sd_g"]), f(inputs["sd_b"]),
                          f(inputs["sd_m"]), f(inputs["sd_v"]), 1e-5)
    sdp = np.zeros((128, 10), np.float32)
    for i in range(5):
        sdp[:, 2 * i] = np.tile(sdsc[i], 2)
        sdp[:, 2 * i + 1] = np.tile(sdbi[i], 2)
    m["sdp4"] = sdp

    bdsc, bdbi = _bn_fold(f(inputs["bnd_g"]), f(inputs["bnd_b"]),
                          f(inputs["bnd_m"]), f(inputs["bnd_v"]), 1e-3)
    bfsc_all, bfbi_all = _bn_fold(f(inputs["bnf_g"]), f(inputs["bnf_b"]),
                                  f(inputs["bnf_m"]), f(inputs["bnf_v"]), 1e-3)
    ch = np.zeros((128, 5 * 6), np.float32)
    t2 = lambda a: np.tile(a, 2)
    for i in range(5):
        cols = [bdsc[i], bdbi[i], f(inputs["prd"])[i],
                bfsc_all[i * C2:(i + 1) * C2], bfbi_all[i * C2:(i + 1) * C2],
                f(inputs["prf"])[i * C2:(i + 1) * C2]]
        for j, v in enumerate(cols):
            ch[:, i * 6 + j] = t2(v)
    m["chainp"] = ch
    return m


_NC_CACHE = {}


def get_nc():
    if "nc" not in _NC_CACHE:
        nc = build()
        nc.compile()
        _NC_CACHE["nc"] = nc
    return _NC_CACHE["nc"]


def kernel(**inputs):
    nc = get_nc()
    shared = prepare_inputs(inputs)
    x = np.asarray(inputs["x"], dtype=np.float32)
    B = x.shape[0]
    in_maps = []
    for b in range(B):
        im = dict(shared)
        im["xin"] = np.ascontiguousarray(x[b]).astype(ml_dtypes.bfloat16)
        in_maps.append(im)
    res = run_bass_kernel_spmd(nc, in_maps, list(range(B)))
    out = np.stack([np.asarray(res.results[b]["yout"], dtype=np.float32)
                    .reshape(5 * C2, OH, OW) for b in range(B)], axis=0)
    return out


def _patch_coresim_prelu():
    """Test-only: CoreSim lacks Prelu (HW has it); emulate via wrapper."""
    import concourse.bass_interp as bi
    import concourse.mybir as mb

    orig = bi.InstructionExecutor.visit_InstActivation

    def visit(self, instruction, *, reg_snapshot=None):
        if instruction.func != mb.ActivationFunctionType.Prelu:
            return orig(self, instruction, reg_snapshot=reg_snapshot)
        from concourse.bass_interp import Direction
        inp = self.view_ap(instruction.ins[0], Direction.READ, instruction,
                           reg_snapshot=reg_snapshot).astype(np.float32)
        def val(arg):
            if hasattr(arg, "value"):
                return arg.value
            v = self.view_ap(arg, Direction.READ, instruction,
                             reg_snapshot=reg_snapshot).astype(np.float32)
            return v.reshape(v.shape[0], -1)
        bias, scale, alpha = (val(instruction.ins[i]) for i in (1, 2, 3))
        inp = inp.reshape(inp.shape[0], -1)
        v = inp * scale + bias
        acted = np.where(v >= 0, v, alpha * v)
        out_view = self.view_ap(instruction.outs[0], Direction.WRITE,
                                instruction, reg_snapshot=reg_snapshot)
        out_view[:] = acted.reshape(out_view.shape).astype(out_view.dtype)

    bi.InstructionExecutor.visit_InstActivation = visit


if __name__ == "__main__":
    # quick CoreSim check of core-0 program against numpy reference
    import reference as ref
    from concourse.bass_interp import CoreSim

    _patch_coresim_prelu()
    inputs = {k: np.asarray(v) for k, v in ref.setup_inputs().items()}
    expected = np.asarray(ref.reference(**inputs))
    nc = build()
    nc.compile()
    shared = prepare_inputs(inputs)
    sim = CoreSim(nc)
    for k, v in shared.items():
        sim.tensor(k)[:] = v
    sim.tensor("xin")[:] = np.asarray(inputs["x"][0]).astype(ml_dtypes.bfloat16)
    sim.simulate()
    got = np.array(sim.tensor("yout")).astype(np.float32).reshape(320, 128, 128)
    e = expected[0]
    err = np.linalg.norm(got - e) / np.linalg.norm(e)
    print("CoreSim core-0 relative error:", err)


# revision 6
# speedup vs baseline: 1.1515x; 1.0326x over previous
"""Trainium2 Bass kernel for nn_DInPBlock (involution block, dense_cnn).

Sharding: pure data parallel - batch dim (8) across 8 NeuronCores, one
image per core. All weights/BN/PReLU params are host-folded and
replicated.

v3 pipeline (per core, image (32,256,256) -> (320,128,128)):
  - BN+PReLU chains run as single Activation-engine Prelu ops with
    per-partition scale/bias/alpha (DVE does only the 9-tap applies)
  - kernel-generation matmuls are block-diagonal full-128-contraction
    PE ops (4x fewer instructions than quadrant-serial)
  - row-shifted apply banks (o1u/o1dn) are persistent pre-zeroed
    buffers; branches processed in descending dilation so the zero
    tails survive; no per-branch zero-fill DMAs
  - apply reads the unshifted o1h tap group first so the single-buffer
    shift DMAs for the next branch overlap the current apply
  - DMA traffic spread across the sync/scalar/gpsimd rings
"""

import numpy as np
import ml_dtypes
from contextlib import ExitStack

import concourse.bass as bass
import concourse.bacc as bacc
import concourse.tile as tile
import concourse.mybir as mybir
from concourse.bass_utils import run_bass_kernel_spmd

F32 = mybir.dt.float32
BF16 = mybir.dt.bfloat16
AF = mybir.ActivationFunctionType
OP = mybir.AluOpType

DILS = (1, 2, 4, 8, 16)
BRANCH_ORDER = (4, 3, 2, 1, 0)  # descending dilation (zero tails survive)
C1, C2, OH, OW = 32, 64, 128, 128
NPIX = OH * OW  # 16384
W2 = 160  # o1h padded width (16 + 128 + 16)
XW = 258  # x bank padded width (1 + 256 + 1)
PO = 129  # odd-column bank width


def _bn_fold(g, b, m, v, eps):
    sc = g / np.sqrt(v + eps)
    return sc.astype(np.float32), (b - m * sc).astype(np.float32)


def build():
    nc = bacc.Bacc("TRN2", target_bir_lowering=False, debug=False)

    x_d = nc.declare_dram_parameter("xin", [C1, 256, 256], BF16, isOutput=False).ap()
    w1red_d = nc.declare_dram_parameter("w1red_bd", [128, 128], BF16, isOutput=False).ap()
    w1span_d = nc.declare_dram_parameter("w1span_bd", [128, 128], BF16, isOutput=False).ap()
    w1init_d = nc.declare_dram_parameter("w1init_bd", [128, 128], BF16, isOutput=False).ap()
    s1p_d = nc.declare_dram_parameter("s1p4", [128, 2], F32, isOutput=False).ap()
    bn1p_d = nc.declare_dram_parameter("bn1p2", [128, 3], F32, isOutput=False).ap()
    wdred_d = nc.declare_dram_parameter("wdred_bd", [128, 5 * 128], BF16, isOutput=False).ap()
    wdspan_d = nc.declare_dram_parameter("wdspan_bd", [128, 5 * 64], BF16, isOutput=False).ap()
    sdp_d = nc.declare_dram_parameter("sdp4", [128, 10], F32, isOutput=False).ap()
    chp_d = nc.declare_dram_parameter("chainp", [128, 5 * 6], F32, isOutput=False).ap()
    y_d = nc.declare_dram_parameter("yout", [5 * C2, NPIX], BF16, isOutput=True).ap()

    with tile.TileContext(nc) as tc, ExitStack() as top:
        pp = top.enter_context(tc.tile_pool(name="params", bufs=1))
        lcx = top.enter_context(tc.tile_pool(name="lcx", bufs=1))
        pmm = top.enter_context(tc.tile_pool(name="pmm", bufs=2, space="PSUM"))
        pms = top.enter_context(tc.tile_pool(name="pms", bufs=2, space="PSUM"))
        dp = top.enter_context(tc.tile_pool(name="dbounce", bufs=2, space="DRAM"))

        w1red_s = pp.tile([128, 128], BF16, tag="w1red")
        w1span_s = pp.tile([128, 128], BF16, tag="w1span")
        w1init_s = pp.tile([128, 128], BF16, tag="w1init")
        nc.gpsimd.dma_start(w1red_s[:], w1red_d[:])
        nc.gpsimd.dma_start(w1span_s[:], w1span_d[:])
        nc.gpsimd.dma_start(w1init_s[:], w1init_d[:])
        s1p_s = pp.tile([128, 2], F32, tag="s1p")
        nc.gpsimd.dma_start(s1p_s[:], s1p_d[:])
        bn1p_s = pp.tile([128, 3], F32, tag="bn1p")
        nc.gpsimd.dma_start(bn1p_s[:], bn1p_d[:])
        wdred_s = pp.tile([128, 5 * 128], BF16, tag="wdred")
        nc.gpsimd.dma_start(wdred_s[:], wdred_d[:])
        wdspan_s = pp.tile([128, 5 * 64], BF16, tag="wdspan")
        nc.gpsimd.dma_start(wdspan_s[:], wdspan_d[:])
        sdp_s = pp.tile([128, 10], F32, tag="sdp")
        nc.gpsimd.dma_start(sdp_s[:], sdp_d[:])
        chp_s = pp.tile([128, 5 * 6], F32, tag="chp")
        nc.gpsimd.dma_start(chp_s[:], chp_d[:])

        # o1 in c-layout: partition (64a + c), free (4096e + 512m + v);
        # pixel quadrant Q = h//32 maps to (a, e) = (Q%2, Q//2).
        o1cx = lcx.tile([128, 8192], BF16, tag="o1cx")

        # ================= invo1 ====================================
        with tc.tile_pool(name="sp1", bufs=1) as sp1:
            o_c = sp1.tile([128, 32 * OW], BF16, tag="o_c")
            r1s = sp1.tile([128, 4096], BF16, tag="r1s")
            kstf1 = sp1.tile([128, 4096], BF16, tag="kstf1")
            k1h = sp1.tile([128, 9, OW], BF16, tag="k1h")
            acc1 = sp1.tile([128, C1, OW], BF16, tag="acc1")
            tmp1 = sp1.tile([128, C1, OW], BF16, tag="tmp1")
            appx_c = sp1.tile([128, 4096], BF16, tag="appx")

            with tc.tile_pool(name="parity", bufs=1) as parp:
                # quadrant-packed c-layout copy of x: partition 32q+c
                # holds x[c, 64q:64q+64, :]
                xq = parp.tile([128, 64 * 256], BF16, tag="xq")
                xdv = x_d.rearrange("c (q rr) w -> q c (rr w)", q=4)
                for q in range(4):
                    nc.sync.dma_start(xq[32 * q:32 * q + 32], xdv[q])
                # row-parity banks (h-layout, padded cols)
                xe = parp.tile([128, C1, XW], BF16, tag="xe")
                xo = parp.tile([128, C1, XW], BF16, tag="xo")
                xv = x_d.rearrange("c (h2 t) w -> t h2 c w", t=2)
                nc.scalar.dma_start(xe[:, :, 1:257], xv[0])
                nc.scalar.dma_start(xo[:, :, 1:257], xv[1])
                nc.gpsimd.memset(xe[:, :, 0:1], 0.0)
                nc.gpsimd.memset(xo[:, :, 0:1], 0.0)
                xeE = parp.tile([128, C1, OW], BF16, tag="xeE")
                xeO = parp.tile([128, C1, PO], BF16, tag="xeO")
                xoE = parp.tile([128, C1, OW], BF16, tag="xoE")
                xoO = parp.tile([128, C1, PO], BF16, tag="xoO")
                xdE = parp.tile([128, C1, OW], BF16, tag="xdE")
                xdO = parp.tile([128, C1, PO], BF16, tag="xdO")

                # 2x2 pool sums from xq (scale folded into w1red)
                t1 = parp.tile([128, 32 * OW], BF16, tag="pt1")
                xqv = xq.rearrange("p (oh2 r ow u) -> p oh2 r ow u",
                                   oh2=32, r=2, ow=128, u=2)
                o_cv = o_c.rearrange("p (a b) -> p a b", a=32)
                t1v = t1.rearrange("p (a b) -> p a b", a=32)
                nc.vector.tensor_tensor(
                    t1v, xqv[:, :, 0, :, 0], xqv[:, :, 0, :, 1], OP.add)
                nc.vector.tensor_tensor(
                    o_cv, xqv[:, :, 1, :, 0], xqv[:, :, 1, :, 1], OP.add)
                nc.vector.tensor_tensor(o_c[:], o_c[:], t1[:], OP.add)

                # column-parity split (strided copies on Act + Pool)
                xev = xe.rearrange("p c (ow t) -> p c ow t", t=2)
                xov = xo.rearrange("p c (ow t) -> p c ow t", t=2)
                nc.scalar.copy(xeO[:], xev[:, :, 0:PO, 0])
                nc.scalar.copy(xeE[:], xev[:, :, 0:OW, 1])
                nc.vector.tensor_copy(xoO[:], xov[:, :, 0:PO, 0])
                nc.vector.tensor_copy(xoE[:], xov[:, :, 0:OW, 1])

                # xd = odd rows shifted down one output row (row -1 = 0)
                nc.vector.memset(xdE[0:1], 0.0)
                nc.vector.memset(xdO[0:1], 0.0)
                nc.sync.dma_start(xdE[1:128], xoE[0:127])
                nc.sync.dma_start(xdO[1:128], xoO[0:127])

                # kgen1 reduce: r1 = relu(bn(w1red_bd @ o_c)) block-diag
                for t in range(4):
                    ps = pmm.tile([128, 1024], F32, tag="mm1")
                    for h in range(2):
                        nc.tensor.matmul(
                            ps[:, 512 * h:512 * (h + 1)],
                            w1red_s[:],
                            o_c[:, 1024 * t + 512 * h:1024 * t + 512 * (h + 1)])
                    nc.scalar.activation(
                        r1s[:, 1024 * t:1024 * (t + 1)], ps[:], AF.Relu,
                        bias=s1p_s[:, 1:2], scale=s1p_s[:, 0:1])

                # kgen1 span: k1 = w1span_bd @ r1 (block-diag); partition
                # block J holds pixel rows [32J, 32J+32)
                for t in range(8):
                    ps2 = pms.tile([128, 512], F32, tag="mm2")
                    nc.tensor.matmul(
                        ps2[:], w1span_s[:], r1s[:, 512 * t:512 * (t + 1)])
                    nc.scalar.copy(kstf1[:, 512 * t:512 * (t + 1)], ps2[:])

                # k1 -> h-layout via DRAM bounce
                kb1 = dp.tile([4, 9, 4096], BF16, tag="kb1", bufs=1)
                for j in range(4):
                    nc.scalar.dma_start(kb1[j], kstf1[32 * j:32 * j + 9])
                for j in range(4):
                    nc.scalar.dma_start(
                        k1h[32 * j:32 * j + 32],
                        kb1[j].rearrange("k (r ow) -> r k ow", r=32))

                # invo1 apply (all taps stride-1)
                banks1 = {0: (xdO, xdE), 1: (xeO, xeE), 2: (xoO, xoE)}
                first = True
                for dy in (1, 2, 0):
                    bO, bE = banks1[dy]
                    taps = (bO[:, :, 0:OW], bE[:, :, 0:OW], bO[:, :, 1:PO])
                    for dx in range(3):
                        k = 3 * dy + dx
                        in1 = k1h[:, k].unsqueeze(1).broadcast_to([128, C1, OW])
                        if first:
                            nc.vector.tensor_tensor(acc1[:], taps[dx], in1, OP.mult)
                            first = False
                        else:
                            nc.vector.tensor_tensor(tmp1[:], taps[dx], in1, OP.mult)
                            nc.vector.tensor_tensor(acc1[:], acc1[:], tmp1[:], OP.add)

            # apply1 output -> c-layout (quadrant packed) via DRAM bounce
            ab = dp.tile([C1, NPIX], BF16, tag="ab", bufs=1)
            nc.gpsimd.dma_start(
                ab.rearrange("c (oh ow) -> oh c ow", ow=OW), acc1[:])
            for q in range(4):
                nc.gpsimd.dma_start(
                    appx_c[32 * q:32 * q + 32], ab[:, 4096 * q:4096 * (q + 1)])

            # init conv (32->64) + bn1 + prelu -> o1cx, all fused on Act
            for e in range(2):
                for m in range(8):
                    ps = pms.tile([128, 512], F32, tag="mm2")
                    nc.tensor.matmul(
                        ps[:], w1init_s[64 * e:64 * e + 64],
                        appx_c[64 * e:64 * e + 64, 512 * m:512 * (m + 1)])
                    nc.scalar.activation(
                        o1cx[:, 4096 * e + 512 * m:4096 * e + 512 * (m + 1)],
                        ps[:], AF.Prelu,
                        bias=bn1p_s[:, 1:2], scale=bn1p_s[:, 0:1],
                        alpha=bn1p_s[:, 2:3])

        # ================= o1h + shifted banks =======================
        lv2 = top.enter_context(tc.tile_pool(name="lv2", bufs=1))
        o1h = lv2.tile([128, C2, W2], BF16, tag="o1h")
        o1u = lv2.tile([128, C2, W2], BF16, tag="o1u")
        o1dn = lv2.tile([128, C2, W2], BF16, tag="o1dn")

        # o1cx -> h-layout via DRAM bounce
        o1b = dp.tile([C2, NPIX], BF16, tag="o1b", bufs=1)
        for a in range(2):
            for e in range(2):
                nc.sync.dma_start(
                    o1b[:, 4096 * (2 * e + a):4096 * (2 * e + a + 1)],
                    o1cx[64 * a:64 * a + 64, 4096 * e:4096 * (e + 1)])
        nc.sync.dma_start(
            o1h[:, :, 16:144], o1b.rearrange("c (oh ow) -> oh c ow", ow=OW))
        # one-time zero pads / tails (Pool engine; branches descend in d)
        nc.gpsimd.memset(o1h[:, :, 0:16], 0.0)
        nc.gpsimd.memset(o1h[:, :, 144:160], 0.0)
        nc.vector.memset(o1u[96:128], 0.0)
        nc.vector.memset(o1dn[0:32], 0.0)

        # ================= branches (software-pipelined) =============
        with tc.tile_pool(name="bk", bufs=1) as bk, \
             tc.tile_pool(name="bkih", bufs=2) as bkih, \
             tc.tile_pool(name="bacc", bufs=1) as ba, \
             tc.tile_pool(name="bch", bufs=1) as bch:

            def stage_kgen(i):
                """kgen for branch i: ris, span, kih transpose."""
                ris = bk.tile([128, 8192], BF16, tag="ris")
                for t in range(8):
                    ps = pmm.tile([128, 1024], F32, tag="mm1")
                    for h in range(2):
                        nc.tensor.matmul(
                            ps[:, 512 * h:512 * (h + 1)],
                            wdred_s[:, 128 * i:128 * (i + 1)],
                            o1cx[:, 1024 * t + 512 * h:1024 * t + 512 * (h + 1)])
                    nc.scalar.activation(
                        ris[:, 1024 * t:1024 * (t + 1)], ps[:], AF.Relu,
                        bias=sdp_s[:, 2 * i + 1:2 * i + 2],
                        scale=sdp_s[:, 2 * i:2 * i + 1])

                kstf = bk.tile([128, 4096], BF16, tag="kstf")
                for m in range(8):
                    ps2 = pms.tile([128, 512], F32, tag="mm2")
                    for e in range(2):
                        nc.tensor.matmul(
                            ps2[64 * e:64 * e + 64],
                            wdspan_s[:, 64 * i:64 * (i + 1)],
                            ris[:, 4096 * e + 512 * m:4096 * e + 512 * (m + 1)],
                            tile_position=(0, 64 * e))
                    nc.scalar.copy(kstf[:, 512 * m:512 * (m + 1)], ps2[:])

                kih = bkih.tile([128, 9, OW], BF16, tag="kih")
                kb = dp.tile([4, 9, 4096], BF16, tag="kb")
                for j in range(4):
                    nc.scalar.dma_start(kb[j], kstf[32 * j:32 * j + 9])
                for j in range(4):
                    nc.scalar.dma_start(
                        kih[32 * j:32 * j + 32],
                        kb[j].rearrange("k (r ow) -> r k ow", r=32))
                return kih

            def stage_shifts(d):
                """refresh the persistent shifted banks for dilation d
                (descending d keeps the zero tails valid)."""
                nc.sync.dma_start(o1dn[d:128], o1h[0:128 - d])
                nc.sync.dma_start(o1u[0:128 - d], o1h[d:128])

            def stage_apply(d, kih):
                """9-tap apply; o1h group first so the single-buffered
                shift DMAs for the next branch overlap this apply."""
                acc = ba.tile([128, C2, OW], BF16, tag="acc", bufs=2)
                tmp = ba.tile([128, C2, OW], BF16, tag="tmp", bufs=1)
                first = True
                for bank, ks in ((o1h, (3, 4, 5)), (o1dn, (0, 1, 2)),
                                 (o1u, (6, 7, 8))):
                    for k in ks:
                        dx = k % 3
                        st = 16 + (dx - 1) * d
                        in0 = bank[:, :, st:st + OW]
                        in1 = kih[:, k].unsqueeze(1).broadcast_to([128, C2, OW])
                        if first:
                            nc.vector.tensor_tensor(acc[:], in0, in1, OP.mult)
                            first = False
                        else:
                            nc.vector.tensor_tensor(tmp[:], in0, in1, OP.mult)
                            nc.vector.tensor_tensor(acc[:], acc[:], tmp[:], OP.add)
                return acc

            def stage_yb(i, acc):
                yb = dp.tile([C2, NPIX], BF16, tag="yb")
                ybv = yb.rearrange("c (hh r ow) -> hh r c ow", hh=2, r=64)
                accv = acc.rearrange("(hh r) c ow -> hh r c ow", hh=2)
                for hh in range(2):
                    nc.gpsimd.dma_start(ybv[hh], accv[hh])
                return yb

            def stage_chain(i, yb):
                """chain = prelu(bnf(prelu(bnd(y)))) as two Act Prelus."""
                cp = [chp_s[:, i * 6 + j:i * 6 + j + 1] for j in range(6)]
                ydv = y_d[i * C2:(i + 1) * C2, :].rearrange(
                    "c (hh g f) -> hh c g f", hh=2, g=2)
                for half in range(2):
                    ya = bch.tile([128, 4096], BF16, tag="ya", bufs=2)
                    tb = bch.tile([128, 4096], BF16, tag="tb", bufs=1)
                    for hh in range(2):
                        nc.scalar.dma_start(
                            ya[64 * hh:64 * hh + 64],
                            yb[:, 8192 * hh + 4096 * half:
                               8192 * hh + 4096 * (half + 1)])
                    nc.scalar.activation(tb[:], ya[:], AF.Prelu,
                                         bias=cp[1], scale=cp[0], alpha=cp[2])
                    nc.scalar.activation(ya[:], tb[:], AF.Prelu,
                                         bias=cp[4], scale=cp[3], alpha=cp[5])
                    yav = ya.rearrange("(hh c) f -> hh c f", hh=2)
                    for hh in range(2):
                        nc.gpsimd.dma_start(ydv[hh, :, half], yav[hh])

            kihs = {}
            ybs = {}
            kihs[0] = stage_kgen(BRANCH_ORDER[0])
            stage_shifts(DILS[BRANCH_ORDER[0]])
            kihs[1] = stage_kgen(BRANCH_ORDER[1])
            for idx in range(5):
                acc = stage_apply(DILS[BRANCH_ORDER[idx]], kihs.pop(idx))
                ybs[idx] = stage_yb(BRANCH_ORDER[idx], acc)
                if idx + 1 < 5:
                    stage_shifts(DILS[BRANCH_ORDER[idx + 1]])
                stage_chain(BRANCH_ORDER[idx], ybs.pop(idx))
                if idx + 2 < 5:
                    kihs[idx + 2] = stage_kgen(BRANCH_ORDER[idx + 2])
    return nc


def prepare_inputs(inputs):
    """Host-side folding of all the small parameters; returns the in_map
    shared structure (everything except per-core x)."""
    f = lambda a: np.asarray(a, dtype=np.float32)
    m = {}
    # block-diagonal weights
    w1red = np.zeros((128, 128), np.float32)
    w1span = np.zeros((128, 128), np.float32)
    wr = f(inputs["w1_red"]).T * 0.25  # [ci, co]
    ws = np.zeros((C1, 32), np.float32)
    ws[:, 0:9] = f(inputs["w1_span"]).T  # [j, k]
    for q in range(4):
        w1red[32 * q:32 * q + 32, 32 * q:32 * q + 32] = wr
        w1span[32 * q:32 * q + 32, 32 * q:32 * q + 32] = ws
    m["w1red_bd"] = w1red.astype(ml_dtypes.bfloat16)
    m["w1span_bd"] = w1span.astype(ml_dtypes.bfloat16)
    w1init = np.zeros((128, 128), np.float32)
    wi = f(inputs["w1_init"]).T  # [ci, co]
    for e in range(2):
        for a in range(2):
            w1init[64 * e + 32 * a:64 * e + 32 * a + 32,
                   64 * a:64 * a + 64] = wi
    m["w1init_bd"] = w1init.astype(ml_dtypes.bfloat16)

    s1sc, s1bi = _bn_fold(f(inputs["s1_g"]), f(inputs["s1_b"]),
                          f(inputs["s1_m"]), f(inputs["s1_v"]), 1e-5)
    m["s1p4"] = np.stack([np.tile(s1sc, 4), np.tile(s1bi, 4)], axis=1)
    sc1, bi1 = _bn_fold(f(inputs["bn1_g"]), f(inputs["bn1_b"]),
                        f(inputs["bn1_m"]), f(inputs["bn1_v"]), 1e-3)
    m["bn1p2"] = np.stack([np.tile(sc1, 2), np.tile(bi1, 2),
                           np.tile(f(inputs["pr1"]), 2)], axis=1)

    wdred = np.zeros((128, 5 * 128), np.float32)
    wdspan = np.zeros((128, 5 * 64), np.float32)
    for i in range(5):
        wrd = f(inputs["wd_red"])[i].T  # [ci, co]
        wsd = np.zeros((C2, 32), np.float32)
        wsd[:, 0:9] = f(inputs["wd_span"])[i].T  # [j, k]
        for a in range(2):
            wdred[64 * a:64 * a + 64, 128 * i + 64 * a:128 * i + 64 * a + 64] = wrd
            wdspan[64 * a:64 * a + 64, 64 * i + 32 * a:64 * i + 32 * a + 32] = wsd
    m["wdred_bd"] = wdred.astype(ml_dtypes.bfloat16)
    m["wdspan_bd"] = wdspan.astype(ml_dtypes.bfloat16)

    sdsc, sdbi = _bn_fold(f(inputs["sd_g"]), f(inputs["sd_b"]),
                          f(inputs["sd_m"]), f(inputs["sd_v"]), 1e-5)
    sdp = np.zeros((128, 10), np.float32)
    for i in range(5):
        sdp[:, 2 * i] = np.tile(sdsc[i], 2)
        sdp[:, 2 * i + 1] = np.tile(sdbi[i], 2)
    m["sdp4"] = sdp

    bdsc, bdbi = _bn_fold(f(inputs["bnd_g"]), f(inputs["bnd_b"]),
                          f(inputs["bnd_m"]), f(inputs["bnd_v"]), 1e-3)
    bfsc_all, bfbi_all = _bn_fold(f(inputs["bnf_g"]), f(inputs["bnf_b"]),
                                  f(inputs["bnf_m"]), f(inputs["bnf_v"]), 1e-3)
    ch = np.zeros((128, 5 * 6), np.float32)
    t2 = lambda a: np.tile(a, 2)
    for i in range(5):
        cols = [bdsc[i], bdbi[i], f(inputs["prd"])[i],
                bfsc_all[i * C2:(i + 1) * C2], bfbi_all[i * C2:(i + 1) * C2],
                f(inputs["prf"])[i * C2:(i + 1) * C2]]
        for j, v in enumerate(cols):
            ch[:, i * 6 + j] = t2(v)
    m["chainp"] = ch
    return m


_NC_CACHE = {}


def get_nc():
    if "nc" not in _NC_CACHE:
        nc = build()
        nc.compile()
        _NC_CACHE["nc"] = nc
    return _NC_CACHE["nc"]


def kernel(**inputs):
    nc = get_nc()
    shared = prepare_inputs(inputs)
    x = np.asarray(inputs["x"], dtype=np.float32)
    B = x.shape[0]
    in_maps = []
    for b in range(B):
        im = dict(shared)
        im["xin"] = np.ascontiguousarray(x[b]).astype(ml_dtypes.bfloat16)
        in_maps.append(im)
    res = run_bass_kernel_spmd(nc, in_maps, list(range(B)))
    out = np.stack([np.asarray(res.results[b]["yout"], dtype=np.float32)
                    .reshape(5 * C2, OH, OW) for b in range(B)], axis=0)
    return out


def _patch_coresim_prelu():
    """Test-only: CoreSim lacks Prelu (HW has it); emulate via wrapper."""
    import concourse.bass_interp as bi
    import concourse.mybir as mb

    orig = bi.InstructionExecutor.visit_InstActivation

    def visit(self, instruction, *, reg_snapshot=None):
        if instruction.func != mb.ActivationFunctionType.Prelu:
            return orig(self, instruction, reg_snapshot=reg_snapshot)
        from concourse.bass_interp import Direction
        inp = self.view_ap(instruction.ins[0], Direction.READ, instruction,
                           reg_snapshot=reg_snapshot).astype(np.float32)
        def val(arg):
            if hasattr(arg, "value"):
                return arg.value
            v = self.view_ap(arg, Direction.READ, instruction,
                             reg_snapshot=reg_snapshot).astype(np.float32)
            return v.reshape(v.shape[0], -1)
        bias, scale, alpha = (val(instruction.ins[i]) for i in (1, 2, 3))
        inp = inp.reshape(inp.shape[0], -1)
        v = inp * scale + bias
        acted = np.where(v >= 0, v, alpha * v)
        out_view = self.view_ap(instruction.outs[0], Direction.WRITE,
                                instruction, reg_snapshot=reg_snapshot)
        out_view[:] = acted.reshape(out_view.shape).astype(out_view.dtype)

    bi.InstructionExecutor.visit_InstActivation = visit


if __name__ == "__main__":
    # quick CoreSim check of core-0 program against numpy reference
    import reference as ref
    from concourse.bass_interp import CoreSim

    _patch_coresim_prelu()
    inputs = {k: np.asarray(v) for k, v in ref.setup_inputs().items()}
    expected = np.asarray(ref.reference(**inputs))
    nc = build()
    nc.compile()
    shared = prepare_inputs(inputs)
    sim = CoreSim(nc)
    for k, v in shared.items():
        sim.tensor(k)[:] = v
    sim.tensor("xin")[:] = np.asarray(inputs["x"][0]).astype(ml_dtypes.bfloat16)
    sim.simulate()
    got = np.array(sim.tensor("yout")).astype(np.float32).reshape(320, 128, 128)
    e = expected[0]
    err = np.linalg.norm(got - e) / np.linalg.norm(e)
    print("CoreSim core-0 relative error:", err)


# revision 9
# speedup vs baseline: 1.2416x; 1.0782x over previous
"""Trainium2 Bass kernel for nn_DInPBlock (involution block, dense_cnn).

Sharding: pure data parallel - batch dim (8) across 8 NeuronCores, one
image per core. All weights/BN/PReLU params are host-folded and
replicated.

v3 pipeline (per core, image (32,256,256) -> (320,128,128)):
  - BN+PReLU chains run as single Activation-engine Prelu ops with
    per-partition scale/bias/alpha (DVE does only the 9-tap applies)
  - kernel-generation matmuls are block-diagonal full-128-contraction
    PE ops (4x fewer instructions than quadrant-serial)
  - row-shifted apply banks (o1u/o1dn) are persistent pre-zeroed
    buffers; branches processed in descending dilation so the zero
    tails survive; no per-branch zero-fill DMAs
  - apply reads the unshifted o1h tap group first so the single-buffer
    shift DMAs for the next branch overlap the current apply
  - DMA traffic spread across the sync/scalar/gpsimd rings
"""

import numpy as np
import ml_dtypes
from contextlib import ExitStack

import concourse.bass as bass
import concourse.bacc as bacc
import concourse.tile as tile
import concourse.mybir as mybir
from concourse.bass_utils import run_bass_kernel_spmd

F32 = mybir.dt.float32
BF16 = mybir.dt.bfloat16
AF = mybir.ActivationFunctionType
OP = mybir.AluOpType

DILS = (1, 2, 4, 8, 16)
BRANCH_ORDER = (4, 3, 2, 1, 0)  # descending dilation (zero tails survive)
C1, C2, OH, OW = 32, 64, 128, 128
NPIX = OH * OW  # 16384
W2 = 160  # o1h padded width (16 + 128 + 16)
XW = 258  # x bank padded width (1 + 256 + 1)
PO = 129  # odd-column bank width


def _bn_fold(g, b, m, v, eps):
    sc = g / np.sqrt(v + eps)
    return sc.astype(np.float32), (b - m * sc).astype(np.float32)


def build():
    nc = bacc.Bacc("TRN2", target_bir_lowering=False, debug=False)

    x_d = nc.declare_dram_parameter("xin", [C1, 256, 256], BF16, isOutput=False).ap()
    w1red_d = nc.declare_dram_parameter("w1red_bd", [128, 128], BF16, isOutput=False).ap()
    w1span_d = nc.declare_dram_parameter("w1span_bd", [128, 128], BF16, isOutput=False).ap()
    w1init_d = nc.declare_dram_parameter("w1init_bd", [128, 128], BF16, isOutput=False).ap()
    s1p_d = nc.declare_dram_parameter("s1p4", [128, 2], F32, isOutput=False).ap()
    bn1p_d = nc.declare_dram_parameter("bn1p2", [128, 3], F32, isOutput=False).ap()
    wdred_d = nc.declare_dram_parameter("wdred_bd", [128, 5 * 128], BF16, isOutput=False).ap()
    wdspan_d = nc.declare_dram_parameter("wdspan_bd", [128, 5 * 64], BF16, isOutput=False).ap()
    sdp_d = nc.declare_dram_parameter("sdp4", [128, 10], F32, isOutput=False).ap()
    chp_d = nc.declare_dram_parameter("chainp", [128, 5 * 6], F32, isOutput=False).ap()
    y_d = nc.declare_dram_parameter("yout", [5 * C2, NPIX], BF16, isOutput=True).ap()

    with tile.TileContext(nc) as tc, ExitStack() as top:
        pp = top.enter_context(tc.tile_pool(name="params", bufs=1))
        lcx = top.enter_context(tc.tile_pool(name="lcx", bufs=1))
        pmm = top.enter_context(tc.tile_pool(name="pmm", bufs=2, space="PSUM"))
        pms = top.enter_context(tc.tile_pool(name="pms", bufs=2, space="PSUM"))
        dp = top.enter_context(tc.tile_pool(name="dbounce", bufs=2, space="DRAM"))

        w1red_s = pp.tile([128, 128], BF16, tag="w1red")
        w1span_s = pp.tile([128, 128], BF16, tag="w1span")
        w1init_s = pp.tile([128, 128], BF16, tag="w1init")
        nc.gpsimd.dma_start(w1red_s[:], w1red_d[:])
        nc.gpsimd.dma_start(w1span_s[:], w1span_d[:])
        nc.gpsimd.dma_start(w1init_s[:], w1init_d[:])
        s1p_s = pp.tile([128, 2], F32, tag="s1p")
        nc.gpsimd.dma_start(s1p_s[:], s1p_d[:])
        bn1p_s = pp.tile([128, 3], F32, tag="bn1p")
        nc.gpsimd.dma_start(bn1p_s[:], bn1p_d[:])
        wdred_s = pp.tile([128, 5 * 128], BF16, tag="wdred")
        nc.gpsimd.dma_start(wdred_s[:], wdred_d[:])
        wdspan_s = pp.tile([128, 5 * 64], BF16, tag="wdspan")
        nc.gpsimd.dma_start(wdspan_s[:], wdspan_d[:])
        sdp_s = pp.tile([128, 10], F32, tag="sdp")
        nc.gpsimd.dma_start(sdp_s[:], sdp_d[:])
        chp_s = pp.tile([128, 5 * 6], F32, tag="chp")
        nc.gpsimd.dma_start(chp_s[:], chp_d[:])

        # o1 in c-layout: partition (64a + c), free (4096e + 512m + v);
        # pixel quadrant Q = h//32 maps to (a, e) = (Q%2, Q//2).
        o1cx = lcx.tile([128, 8192], BF16, tag="o1cx")

        # ================= invo1 ====================================
        with tc.tile_pool(name="sp1", bufs=1) as sp1:
            o_c = sp1.tile([128, 32 * OW], BF16, tag="o_c")
            r1s = sp1.tile([128, 4096], BF16, tag="r1s")
            kstf1 = sp1.tile([128, 4096], BF16, tag="kstf1")
            k1h = sp1.tile([128, 9, OW], BF16, tag="k1h")
            acc1 = sp1.tile([128, C1, OW], BF16, tag="acc1")
            tmp1 = sp1.tile([128, C1, OW], BF16, tag="tmp1")
            appx_c = sp1.tile([128, 4096], BF16, tag="appx")

            with tc.tile_pool(name="parity", bufs=1) as parp:
                # quadrant-packed c-layout copy of x: partition 32q+c
                # holds x[c, 64q:64q+64, :]
                xq = parp.tile([128, 64 * 256], BF16, tag="xq")
                xdv = x_d.rearrange("c (q rr) w -> q c (rr w)", q=4)
                # row-parity banks (h-layout, padded cols)
                xe = parp.tile([128, C1, XW], BF16, tag="xe")
                xo = parp.tile([128, C1, XW], BF16, tag="xo")
                xv = x_d.rearrange("c (h2 t) w -> t h2 c w", t=2)
                nc.sync.dma_start(xo[:, :, 1:257], xv[1])
                nc.scalar.dma_start(xe[:, :, 1:257], xv[0])
                for q in range(4):
                    nc.sync.dma_start(xq[32 * q:32 * q + 32], xdv[q])
                nc.gpsimd.memset(xe[:, :, 0:1], 0.0)
                nc.gpsimd.memset(xo[:, :, 0:1], 0.0)
                xeE = parp.tile([128, C1, OW], BF16, tag="xeE")
                xeO = parp.tile([128, C1, PO], BF16, tag="xeO")
                xoE = parp.tile([128, C1, OW], BF16, tag="xoE")
                xoO = parp.tile([128, C1, PO], BF16, tag="xoO")
                xdE = parp.tile([128, C1, OW], BF16, tag="xdE")
                xdO = parp.tile([128, C1, PO], BF16, tag="xdO")

                # 2x2 pool sums from xq (scale folded into w1red)
                t1 = parp.tile([128, 32 * OW], BF16, tag="pt1")
                xqv = xq.rearrange("p (oh2 r ow u) -> p oh2 r ow u",
                                   oh2=32, r=2, ow=128, u=2)
                o_cv = o_c.rearrange("p (a b) -> p a b", a=32)
                t1v = t1.rearrange("p (a b) -> p a b", a=32)
                nc.vector.tensor_tensor(
                    t1v, xqv[:, :, 0, :, 0], xqv[:, :, 0, :, 1], OP.add)
                nc.vector.tensor_tensor(
                    o_cv, xqv[:, :, 1, :, 0], xqv[:, :, 1, :, 1], OP.add)
                nc.vector.tensor_tensor(o_c[:], o_c[:], t1[:], OP.add)

                # column-parity split (strided copies on Act + Pool)
                xev = xe.rearrange("p c (ow t) -> p c ow t", t=2)
                xov = xo.rearrange("p c (ow t) -> p c ow t", t=2)
                nc.scalar.copy(xeO[:], xev[:, :, 0:PO, 0])
                nc.scalar.copy(xeE[:], xev[:, :, 0:OW, 1])
                nc.vector.tensor_copy(xoO[:], xov[:, :, 0:PO, 0])
                nc.vector.tensor_copy(xoE[:], xov[:, :, 0:OW, 1])

                # xd = odd rows shifted down one output row (row -1 = 0)
                nc.vector.memset(xdE[0:1], 0.0)
                nc.vector.memset(xdO[0:1], 0.0)
                nc.sync.dma_start(xdE[1:128], xoE[0:127])
                nc.sync.dma_start(xdO[1:128], xoO[0:127])

                # kgen1 reduce: r1 = relu(bn(w1red_bd @ o_c)) block-diag
                for t in range(4):
                    ps = pmm.tile([128, 1024], F32, tag="mm1")
                    for h in range(2):
                        nc.tensor.matmul(
                            ps[:, 512 * h:512 * (h + 1)],
                            w1red_s[:],
                            o_c[:, 1024 * t + 512 * h:1024 * t + 512 * (h + 1)])
                    nc.scalar.activation(
                        r1s[:, 1024 * t:1024 * (t + 1)], ps[:], AF.Relu,
                        bias=s1p_s[:, 1:2], scale=s1p_s[:, 0:1])

                # kgen1 span: k1 = w1span_bd @ r1 (block-diag); partition
                # block J holds pixel rows [32J, 32J+32)
                for t in range(8):
                    ps2 = pms.tile([128, 512], F32, tag="mm2")
                    nc.tensor.matmul(
                        ps2[:], w1span_s[:], r1s[:, 512 * t:512 * (t + 1)])
                    nc.scalar.copy(kstf1[:, 512 * t:512 * (t + 1)], ps2[:])

                # k1 -> h-layout via DRAM bounce
                kb1 = dp.tile([4, 9, 4096], BF16, tag="kb1", bufs=1)
                for j in range(4):
                    nc.scalar.dma_start(kb1[j], kstf1[32 * j:32 * j + 9])
                for j in range(4):
                    nc.scalar.dma_start(
                        k1h[32 * j:32 * j + 32],
                        kb1[j].rearrange("k (r ow) -> r k ow", r=32))

                # invo1 apply (all taps stride-1)
                banks1 = {0: (xdO, xdE), 1: (xeO, xeE), 2: (xoO, xoE)}
                first = True
                for dy in (1, 2, 0):
                    bO, bE = banks1[dy]
                    taps = (bO[:, :, 0:OW], bE[:, :, 0:OW], bO[:, :, 1:PO])
                    for dx in range(3):
                        k = 3 * dy + dx
                        in1 = k1h[:, k].unsqueeze(1).broadcast_to([128, C1, OW])
                        if first:
                            nc.vector.tensor_tensor(acc1[:], taps[dx], in1, OP.mult)
                            first = False
                        else:
                            nc.vector.tensor_tensor(tmp1[:], taps[dx], in1, OP.mult)
                            nc.vector.tensor_tensor(acc1[:], acc1[:], tmp1[:], OP.add)

            # apply1 output -> c-layout (quadrant packed) via DRAM bounce
            ab = dp.tile([C1, NPIX], BF16, tag="ab", bufs=1)
            nc.gpsimd.dma_start(
                ab.rearrange("c (oh ow) -> oh c ow", ow=OW), acc1[:])
            for q in range(4):
                nc.gpsimd.dma_start(
                    appx_c[32 * q:32 * q + 32], ab[:, 4096 * q:4096 * (q + 1)])

            # init conv (32->64) + bn1 + prelu -> o1cx, all fused on Act
            for e in range(2):
                for m in range(8):
                    ps = pms.tile([128, 512], F32, tag="mm2")
                    nc.tensor.matmul(
                        ps[:], w1init_s[64 * e:64 * e + 64],
                        appx_c[64 * e:64 * e + 64, 512 * m:512 * (m + 1)])
                    nc.scalar.activation(
                        o1cx[:, 4096 * e + 512 * m:4096 * e + 512 * (m + 1)],
                        ps[:], AF.Prelu,
                        bias=bn1p_s[:, 1:2], scale=bn1p_s[:, 0:1],
                        alpha=bn1p_s[:, 2:3])

        # ================= o1h + shifted banks =======================
        lv2 = top.enter_context(tc.tile_pool(name="lv2", bufs=1))
        o1h = lv2.tile([128, C2, W2], BF16, tag="o1h")
        o1uA = lv2.tile([128, C2, W2], BF16, tag="o1uA")
        o1uB = lv2.tile([128, C2, W2], BF16, tag="o1uB")
        o1dnA = lv2.tile([128, C2, W2], BF16, tag="o1dnA")
        o1dnB = lv2.tile([128, C2, W2], BF16, tag="o1dnB")
        o1us = [o1uA, o1uB]
        o1dns = [o1dnA, o1dnB]

        # o1cx -> h-layout via DRAM bounce
        o1b = dp.tile([C2, NPIX], BF16, tag="o1b", bufs=1)
        for a in range(2):
            for e in range(2):
                nc.sync.dma_start(
                    o1b[:, 4096 * (2 * e + a):4096 * (2 * e + a + 1)],
                    o1cx[64 * a:64 * a + 64, 4096 * e:4096 * (e + 1)])
        nc.sync.dma_start(
            o1h[:, :, 16:144], o1b.rearrange("c (oh ow) -> oh c ow", ow=OW))
        # one-time zero pads / tails (Pool engine; branches descend in d)
        nc.gpsimd.memset(o1h[:, :, 0:16], 0.0)
        nc.gpsimd.memset(o1h[:, :, 144:160], 0.0)
        for b in range(2):
            nc.vector.memset(o1us[b][96:128], 0.0)
            nc.vector.memset(o1dns[b][0:32], 0.0)

        # ================= branches (software-pipelined) =============
        with tc.tile_pool(name="bk", bufs=1) as bk, \
             tc.tile_pool(name="bkih", bufs=2) as bkih, \
             tc.tile_pool(name="bacc", bufs=1) as ba, \
             tc.tile_pool(name="bch", bufs=1) as bch:

            def stage_kgen(i):
                """kgen for branch i: ris, span, kih transpose."""
                ris = bk.tile([128, 8192], BF16, tag="ris")
                for t in range(8):
                    ps = pmm.tile([128, 1024], F32, tag="mm1")
                    for h in range(2):
                        nc.tensor.matmul(
                            ps[:, 512 * h:512 * (h + 1)],
                            wdred_s[:, 128 * i:128 * (i + 1)],
                            o1cx[:, 1024 * t + 512 * h:1024 * t + 512 * (h + 1)])
                    nc.scalar.activation(
                        ris[:, 1024 * t:1024 * (t + 1)], ps[:], AF.Relu,
                        bias=sdp_s[:, 2 * i + 1:2 * i + 2],
                        scale=sdp_s[:, 2 * i:2 * i + 1])

                kstf = bk.tile([128, 4096], BF16, tag="kstf")
                for m in range(8):
                    ps2 = pms.tile([128, 512], F32, tag="mm2")
                    for e in range(2):
                        nc.tensor.matmul(
                            ps2[64 * e:64 * e + 64],
                            wdspan_s[:, 64 * i:64 * (i + 1)],
                            ris[:, 4096 * e + 512 * m:4096 * e + 512 * (m + 1)],
                            tile_position=(0, 64 * e))
                    nc.scalar.copy(kstf[:, 512 * m:512 * (m + 1)], ps2[:])

                kih = bkih.tile([128, 9, OW], BF16, tag="kih")
                kb = dp.tile([4, 9, 4096], BF16, tag="kb")
                for j in range(4):
                    nc.scalar.dma_start(kb[j], kstf[32 * j:32 * j + 9])
                for j in range(4):
                    nc.scalar.dma_start(
                        kih[32 * j:32 * j + 32],
                        kb[j].rearrange("k (r ow) -> r k ow", r=32))
                return kih

            def stage_shifts(idx):
                """refresh the shifted banks for branch idx (descending d
                keeps each buffer's zero tails valid)."""
                d = DILS[BRANCH_ORDER[idx]]
                o1dn, o1u = o1dns[idx % 2], o1us[idx % 2]
                nc.sync.dma_start(o1dn[d:128], o1h[0:128 - d])
                nc.sync.dma_start(o1u[0:128 - d], o1h[d:128])

            def stage_apply(idx, kih):
                """9-tap apply; dn/u groups first so the next branch's
                shift DMAs overlap this apply's tail."""
                d = DILS[BRANCH_ORDER[idx]]
                o1dn, o1u = o1dns[idx % 2], o1us[idx % 2]
                acc = ba.tile([128, C2, OW], BF16, tag="acc", bufs=2)
                tmp = ba.tile([128, C2 // 2, OW], BF16, tag="tmp", bufs=1)
                first = True
                for bank, ks in ((o1dn, (0, 1, 2)), (o1u, (6, 7, 8)),
                                 (o1h, (3, 4, 5))):
                    for k in ks:
                        dx = k % 3
                        st = 16 + (dx - 1) * d
                        in0 = bank[:, :, st:st + OW]
                        in1 = kih[:, k].unsqueeze(1).broadcast_to([128, C2, OW])
                        if first:
                            nc.vector.tensor_tensor(acc[:], in0, in1, OP.mult)
                            first = False
                            continue
                        for ch in range(2):
                            cs = slice(32 * ch, 32 * ch + 32)
                            nc.vector.tensor_tensor(
                                tmp[:], in0[:, cs], in1[:, cs], OP.mult)
                            nc.vector.tensor_tensor(
                                acc[:, cs], acc[:, cs], tmp[:], OP.add)
                return acc

            def stage_yb(i, acc):
                yb = dp.tile([C2, NPIX], BF16, tag="yb")
                ybv = yb.rearrange("c (hh r ow) -> hh r c ow", hh=2, r=64)
                accv = acc.rearrange("(hh r) c ow -> hh r c ow", hh=2)
                for hh in range(2):
                    nc.gpsimd.dma_start(ybv[hh], accv[hh])
                return yb

            def stage_chain(i, yb):
                """chain = prelu(bnf(prelu(bnd(y)))) as two Act Prelus,
                quarter-granular for a short pipeline tail."""
                cp = [chp_s[:, i * 6 + j:i * 6 + j + 1] for j in range(6)]
                ydv = y_d[i * C2:(i + 1) * C2, :].rearrange(
                    "c (hh g f) -> hh c g f", hh=2, g=4)
                for g in range(4):
                    ya = bch.tile([128, 2048], BF16, tag="ya", bufs=2)
                    tb = bch.tile([128, 2048], BF16, tag="tb", bufs=1)
                    for hh in range(2):
                        nc.gpsimd.dma_start(
                            ya[64 * hh:64 * hh + 64],
                            yb[:, 8192 * hh + 2048 * g:
                               8192 * hh + 2048 * (g + 1)])
                    nc.scalar.activation(tb[:], ya[:], AF.Prelu,
                                         bias=cp[1], scale=cp[0], alpha=cp[2])
                    nc.scalar.activation(ya[:], tb[:], AF.Prelu,
                                         bias=cp[4], scale=cp[3], alpha=cp[5])
                    yav = ya.rearrange("(hh c) f -> hh c f", hh=2)
                    for hh in range(2):
                        nc.gpsimd.dma_start(ydv[hh, :, g], yav[hh])

            kihs = {}
            ybs = {}
            kihs[0] = stage_kgen(BRANCH_ORDER[0])
            stage_shifts(0)
            kihs[1] = stage_kgen(BRANCH_ORDER[1])
            stage_shifts(1)
            for idx in range(5):
                acc = stage_apply(idx, kihs.pop(idx))
                ybs[idx] = stage_yb(BRANCH_ORDER[idx], acc)
                if idx + 2 < 5:
                    stage_shifts(idx + 2)
                stage_chain(BRANCH_ORDER[idx], ybs.pop(idx))
                if idx + 2 < 5:
                    kihs[idx + 2] = stage_kgen(BRANCH_ORDER[idx + 2])
    return nc


def prepare_inputs(inputs):
    """Host-side folding of all the small parameters; returns the in_map
    shared structure (everything except per-core x)."""
    f = lambda a: np.asarray(a, dtype=np.float32)
    m = {}
    # block-diagonal weights
    w1red = np.zeros((128, 128), np.float32)
    w1span = np.zeros((128, 128), np.float32)
    wr = f(inputs["w1_red"]).T * 0.25  # [ci, co]
    ws = np.zeros((C1, 32), np.float32)
    ws[:, 0:9] = f(inputs["w1_span"]).T  # [j, k]
    for q in range(4):
        w1red[32 * q:32 * q + 32, 32 * q:32 * q + 32] = wr
        w1span[32 * q:32 * q + 32, 32 * q:32 * q + 32] = ws
    m["w1red_bd"] = w1red.astype(ml_dtypes.bfloat16)
    m["w1span_bd"] = w1span.astype(ml_dtypes.bfloat16)
    w1init = np.zeros((128, 128), np.float32)
    wi = f(inputs["w1_init"]).T  # [ci, co]
    for e in range(2):
        for a in range(2):
            w1init[64 * e + 32 * a:64 * e + 32 * a + 32,
                   64 * a:64 * a + 64] = wi
    m["w1init_bd"] = w1init.astype(ml_dtypes.bfloat16)

    s1sc, s1bi = _bn_fold(f(inputs["s1_g"]), f(inputs["s1_b"]),
                          f(inputs["s1_m"]), f(inputs["s1_v"]), 1e-5)
    m["s1p4"] = np.stack([np.tile(s1sc, 4), np.tile(s1bi, 4)], axis=1)
    sc1, bi1 = _bn_fold(f(inputs["bn1_g"]), f(inputs["bn1_b"]),
                        f(inputs["bn1_m"]), f(inputs["bn1_v"]), 1e-3)
    m["bn1p2"] = np.stack([np.tile(sc1, 2), np.tile(bi1, 2),
                           np.tile(f(inputs["pr1"]), 2)], axis=1)

    wdred = np.zeros((128, 5 * 128), np.float32)
    wdspan = np.zeros((128, 5 * 64), np.float32)
    for i in range(5):
        wrd = f(inputs["wd_red"])[i].T  # [ci, co]
        wsd = np.zeros((C2, 32), np.float32)
        wsd[:, 0:9] = f(inputs["wd_span"])[i].T  # [j, k]
        for a in range(2):
            wdred[64 * a:64 * a + 64, 128 * i + 64 * a:128 * i + 64 * a + 64] = wrd
            wdspan[64 * a:64 * a + 64, 64 * i + 32 * a:64 * i + 32 * a + 32] = wsd
    m["wdred_bd"] = wdred.astype(ml_dtypes.bfloat16)
    m["wdspan_bd"] = wdspan.astype(ml_dtypes.bfloat16)

    sdsc, sdbi = _bn_fold(f(inputs["sd_g"]), f(inputs["sd_b"]),
                          f(inputs["sd_m"]), f(inputs["sd_v"]), 1e-5)
    sdp = np.zeros((128, 10), np.float32)
    for i in range(5):
        sdp[:, 2 * i] = np.tile(sdsc[i], 2)
        sdp[:, 2 * i + 1] = np.tile(sdbi[i], 2)
    m["sdp4"] = sdp

    bdsc, bdbi = _bn_fold(f(inputs["bnd_g"]), f(inputs["bnd_b"]),
                          f(inputs["bnd_m"]), f(inputs["bnd_v"]), 1e-3)
    bfsc_all, bfbi_all = _bn_fold(f(inputs["bnf_g"]), f(inputs["bnf_b"]),
                                  f(inputs["bnf_m"]), f(inputs["bnf_v"]), 1e-3)
    ch = np.zeros((128, 5 * 6), np.float32)
    t2 = lambda a: np.tile(a, 2)
    for i in range(5):
        cols = [bdsc[i], bdbi[i], f(inputs["prd"])[i],
                bfsc_all[i * C2:(i + 1) * C2], bfbi_all[i * C2:(i + 1) * C2],
                f(inputs["prf"])[i * C2:(i + 1) * C2]]
        for j, v in enumerate(cols):
            ch[:, i * 6 + j] = t2(v)
    m["chainp"] = ch
    return m


_NC_CACHE = {}


def get_nc():
    if "nc" not in _NC_CACHE:
        nc = build()
        nc.compile()
        _NC_CACHE["nc"] = nc
    return _NC_CACHE["nc"]


def kernel(**inputs):
    nc = get_nc()
    shared = prepare_inputs(inputs)
    x = np.asarray(inputs["x"], dtype=np.float32)
    B = x.shape[0]
    in_maps = []
    for b in range(B):
        im = dict(shared)
        im["xin"] = np.ascontiguousarray(x[b]).astype(ml_dtypes.bfloat16)
        in_maps.append(im)
    res = run_bass_kernel_spmd(nc, in_maps, list(range(B)))
    out = np.stack([np.asarray(res.results[b]["yout"], dtype=np.float32)
                    .reshape(5 * C2, OH, OW) for b in range(B)], axis=0)
    return out


def _patch_coresim_prelu():
    """Test-only: CoreSim lacks Prelu (HW has it); emulate via wrapper."""
    import concourse.bass_interp as bi
    import concourse.mybir as mb

    orig = bi.InstructionExecutor.visit_InstActivation

    def visit(self, instruction, *, reg_snapshot=None):
        if instruction.func != mb.ActivationFunctionType.Prelu:
            return orig(self, instruction, reg_snapshot=reg_snapshot)
        from concourse.bass_interp import Direction
        inp = self.view_ap(instruction.ins[0], Direction.READ, instruction,
                           reg_snapshot=reg_snapshot).astype(np.float32)
        def val(arg):
            if hasattr(arg, "value"):
                return arg.value
            v = self.view_ap(arg, Direction.READ, instruction,
                             reg_snapshot=reg_snapshot).astype(np.float32)
            return v.reshape(v.shape[0], -1)
        bias, scale, alpha = (val(instruction.ins[i]) for i in (1, 2, 3))
        inp = inp.reshape(inp.shape[0], -1)
        v = inp * scale + bias
        acted = np.where(v >= 0, v, alpha * v)
        out_view = self.view_ap(instruction.outs[0], Direction.WRITE,
                                instruction, reg_snapshot=reg_snapshot)
        out_view[:] = acted.reshape(out_view.shape).astype(out_view.dtype)

    bi.InstructionExecutor.visit_InstActivation = visit


if __name__ == "__main__":
    # quick CoreSim check of core-0 program against numpy reference
    import reference as ref
    from concourse.bass_interp import CoreSim

    _patch_coresim_prelu()
    inputs = {k: np.asarray(v) for k, v in ref.setup_inputs().items()}
    expected = np.asarray(ref.reference(**inputs))
    nc = build()
    nc.compile()
    shared = prepare_inputs(inputs)
    sim = CoreSim(nc)
    for k, v in shared.items():
        sim.tensor(k)[:] = v
    sim.tensor("xin")[:] = np.asarray(inputs["x"][0]).astype(ml_dtypes.bfloat16)
    sim.simulate()
    got = np.array(sim.tensor("yout")).astype(np.float32).reshape(320, 128, 128)
    e = expected[0]
    err = np.linalg.norm(got - e) / np.linalg.norm(e)
    print("CoreSim core-0 relative error:", err)
